# revision 2
# baseline (speedup 1.0000x reference)
"""Trainium2 Bass kernel v2 for segment-causal GQA attention.

Sharding: 8 cores = batch (2) x kv-head (4), as baseline. All matmul
operands bf16 (1 cyc/row at any free size), halving DMA and enabling
128-wide attention t-tiles. RMS scales folded into host rope tables;
SCALE*rstd_k folded into kTn columns so exp batches over s-tiles.
Attention: per (g, t-tile) unit = n_e logit matmuls into one psum run,
one exp, one mask mult, n_e den + n_e qkv matmuls; den/qkv batched
per (g, 4-tt quad) for one reciprocal + broadcast + normalize.
"""

import sys

sys.path.insert(0, "/opt/trn_rl_repo")

import numpy as np

import concourse.bacc as bacc
import concourse.bass as bass  # noqa: F401
import concourse.tile as tile
from concourse import mybir
from concourse.bass_utils import run_bass_kernel_spmd

B, T, D = 2, 1024, 2048
N, K, H = 16, 4, 128
G = N // K
EPS = 1e-6
ROPE_BASE = 10000.0
ND = D // 128        # 16 d-tiles
NS = T // 128        # 8 s-tiles
NT = T // 128        # 8 t-tiles
F32 = mybir.dt.float32
F32R = mybir.dt.float32r
BF16 = mybir.dt.bfloat16
EXPF = mybir.ActivationFunctionType.Exp
SQRTF = mybir.ActivationFunctionType.Sqrt

LAST_RESULTS = None  # test harness reads exec_time_ns from here


def _positions(seg):
    t = seg.shape[0]
    idx = np.arange(t, dtype=np.int64)
    is_start = np.concatenate([[True], seg[1:] != seg[:-1]])
    seg_start = np.maximum.accumulate(np.where(is_start, idx, 0))
    return (idx - seg_start).astype(np.float64)


def _classify(seg_rows):
    """Union tile classification over batches at 128x128 granularity.

    Returns (plan, masks_per_batch): plan[tt] = list of (si, kind, mask_idx);
    masks_per_batch[b] = float32 [max(n_masks,1), 128, 128] of 0/1.
    """
    idx = np.arange(T)
    valids = []
    for b in range(len(seg_rows)):
        seg = seg_rows[b]
        valids.append((seg[:, None] == seg[None, :]) & (idx[:, None] <= idx[None, :]))
    plan = []
    mask_list = [[] for _ in range(len(seg_rows))]
    n_masks = 0
    for tt in range(NT):
        t0 = tt * 128
        entries = []
        for si in range(NS):
            s0 = si * 128
            subs = [v[s0:s0 + 128, t0:t0 + 128] for v in valids]
            if not any(s.any() for s in subs):
                continue
            if all(s.all() for s in subs):
                entries.append((si, "full", -1))
            else:
                for b in range(len(seg_rows)):
                    mask_list[b].append(subs[b].astype(np.float32))
                entries.append((si, "partial", n_masks))
                n_masks += 1
        plan.append(entries)
    masks = []
    for b in range(len(seg_rows)):
        if n_masks:
            masks.append(np.ascontiguousarray(np.stack(mask_list[b]), np.float32))
        else:
            masks.append(np.zeros((1, 128, 128), np.float32))
    return plan, masks


def _build_nc(plan, n_masks):
    from contextlib import ExitStack

    nc = bacc.Bacc(None, target_bir_lowering=False, debug=False)
    MULT = mybir.AluOpType.mult

    xt_d = nc.dram_tensor("xt", [128, ND * T], BF16, kind="ExternalInput")
    wq0_d = nc.dram_tensor("wq0", [128, 2 * D], BF16, kind="ExternalInput")
    wq1_d = nc.dram_tensor("wq1", [128, 2 * D], BF16, kind="ExternalInput")
    wkv_d = nc.dram_tensor("wkv", [128, 2 * D], BF16, kind="ExternalInput")
    wo_d = nc.dram_tensor("wo", [128, G * D], BF16, kind="ExternalInput")
    rtq_d = nc.dram_tensor("rtq", [128, 4 * T], BF16, kind="ExternalInput")
    rtk_d = nc.dram_tensor("rtk", [128, 4 * T], BF16, kind="ExternalInput")
    nm = max(n_masks, 1)
    mpk_d = nc.dram_tensor("mpk", [128, nm * 128], BF16, kind="ExternalInput")
    iden_d = nc.dram_tensor("iden", [128, 128], BF16, kind="ExternalInput")
    cstb_d = nc.dram_tensor("cstb", [128, 4], BF16, kind="ExternalInput")
    cst_d = nc.dram_tensor("cst", [128, 2], F32, kind="ExternalInput")
    bc_d = nc.dram_tensor("bcw", [2, 128], F32R, kind="ExternalInput")
    out_d = nc.dram_tensor("out", [T, D], BF16, kind="ExternalOutput")

    es = ExitStack()
    with es:
        es.enter_context(nc.allow_low_precision("bf16 kernel"))
        tc = es.enter_context(tile.TileContext(nc))
        pool = lambda *a, **k: es.enter_context(tc.tile_pool(*a, **k))
        pp = pool(name="persist", bufs=1)

        # ---------------- persistent SBUF tiles ----------------
        xt = pp.tile([128, ND * T], BF16, tag="xt", name="xt")
        wq0 = pp.tile([128, 2 * D], BF16, tag="wq0", name="wq0")
        wq1 = pp.tile([128, 2 * D], BF16, tag="wq1", name="wq1")
        wkv = pp.tile([128, 2 * D], BF16, tag="wkv", name="wkv")
        wo = pp.tile([128, G * D], BF16, tag="wo", name="wo")
        rtq = pp.tile([128, 4 * T], BF16, tag="rtq", name="rtq")
        rtk = pp.tile([128, 4 * T], BF16, tag="rtk", name="rtk")
        mpk = pp.tile([128, nm * 128], BF16, tag="mpk", name="mpk")
        iden = pp.tile([128, 128], BF16, tag="iden", name="iden")
        cstb = pp.tile([128, 4], BF16, tag="cstb", name="cstb")
        cst = pp.tile([128, 2], F32, tag="cst", name="cst")
        bcw = pp.tile([2, 128], F32R, tag="bcw", name="bcw")
        qh = [pp.tile([128, T], BF16, tag=f"qh{g}", name=f"qh{g}") for g in range(G)]
        kTn = pp.tile([128, T], BF16, tag="kTn", name="kTn")
        V = pp.tile([128, T], BF16, tag="V", name="V")
        vt_sb = pp.tile([128, T], BF16, tag="vt", name="vt")
        sqk = pp.tile([128, T], BF16, tag="sqk", name="sqk")
        qkvh = [pp.tile([128, T], BF16, tag=f"qkvh{g}", name=f"qkvh{g}")
                for g in range(G)]
        rk = pp.tile([1, T], F32, tag="rk", name="rk")
        rkb = pp.tile([128, T], F32, tag="rkb", name="rkb")

        onesb = cstb[:, 2:3]          # bf16 ones column (den / ksum lhsT)

        # ---------------- DMAs (sync queue, priority order) ----------------
        def xchunk(ci):
            sl = slice(ci * 2 * T, (ci + 1) * 2 * T)
            nc.sync.dma_start(xt[:, sl], xt_d[:, sl])

        nc.sync.dma_start(xt[:, 0:T], xt_d[:, 0:T])                # di0
        nc.sync.dma_start(wq0[:, 0:512], wq0_d[:, 0:512])          # f0 di0-3
        nc.sync.dma_start(xt[:, T:2 * T], xt_d[:, T:2 * T])        # di1
        nc.sync.dma_start(wq0[:, D:D + 512], wq0_d[:, D:D + 512])  # f2 di0-3
        xchunk(1)
        nc.sync.dma_start(wq0[:, 512:D], wq0_d[:, 512:D])          # f0 di4-15
        xchunk(2)
        nc.sync.dma_start(wq0[:, D + 512:2 * D], wq0_d[:, D + 512:2 * D])
        xchunk(3)
        for t_, d_ in [(cstb, cstb_d), (cst, cst_d), (bcw, bc_d),
                       (iden, iden_d)]:
            nc.sync.dma_start(t_[:], d_[:])
        for ci in range(4, 8):
            xchunk(ci)
        nc.sync.dma_start(wq1[:], wq1_d[:])
        nc.sync.dma_start(rtq[:], rtq_d[:])
        nc.sync.dma_start(wkv[:], wkv_d[:])
        nc.sync.dma_start(rtk[:], rtk_d[:])
        nc.sync.dma_start(wo[:], wo_d[:])
        nc.sync.dma_start(mpk[:], mpk_d[:])

        # ---------------- stream pools ----------------
        sbra = pool(name="sb_ra", bufs=4)          # rope outputs ra/rb
        sbb = pool(name="sb_bps", bufs=2)          # bps broadcast sbuf
        sbP = pool(name="sb_P", bufs=8)            # attention P tiles
        es1 = ExitStack()
        pool1 = lambda *a, **k: es1.enter_context(tc.tile_pool(*a, **k))
        sbpc = pool1(name="sb_pc", bufs=6)         # psum->sbuf proj copies
        sbm = pool1(name="sb_m", bufs=2)           # rope temporaries
        sbq = pool1(name="sb_sq", bufs=4)          # square tiles
        sbr = pool1(name="sb_rstd", bufs=2)        # rstd / stmp tiles

        psproj = pool1(name="ps_proj", bufs=4, space="PSUM")
        ps_sm = pool1(name="ps_small", bufs=2, space="PSUM")
        ps_bps = pool1(name="ps_bps", bufs=1, space="PSUM")

        # PE p-state warmup: matmuls on an uninitialized scratch tile (values
        # never read) while the input DMAs stream in
        wsc = sbm.tile([128, 128], BF16, tag="m1", name="wscratch")
        nc.vector.memset(wsc[:], 0.0)
        warm = psproj.tile([128, 512], F32, tag="proj", name="warm")
        for _ in range(18):
            nc.tensor.matmul(warm[:, 0:128], wsc[:], wsc[:],
                             start=True, stop=True)
        wdmy = sbr.tile([1, 2], F32, tag="dmy", name="wdmy")
        nc.scalar.copy(wdmy[:], warm[0:1, 0:2])

        def proj_pass(w, feats, inserts=None, fillers=0):
            """d-outer pass over `feats` = list of (col_off, psum pair).
            inserts: {di: fn} PE-stream injections. fillers: p-state keepalive
            matmuls per di boundary while the xt stream is still arriving."""
            pss = {}
            for fo, _ in feats:
                for c in range(2):
                    pss[(fo, c)] = psproj.tile([128, 512], F32, tag="proj",
                                               name="proj")
            for di in range(ND):
                if inserts and di in inserts:
                    inserts[di]()
                if fillers and di < 8:
                    for _ in range(fillers):
                        nc.tensor.matmul(warm[:, 128:256], wsc[:], wsc[:],
                                         start=True, stop=True)
                for fo, _ in feats:
                    for c in range(2):
                        nc.tensor.matmul(
                            pss[(fo, c)][:],
                            w[:, fo + di * 128: fo + (di + 1) * 128],
                            xt[:, di * T + c * 512: di * T + (c + 1) * 512],
                            start=(di == 0), stop=(di == ND - 1))
            return pss

        def rope_q(pca, pcb, ra, rb, cs):
            m1 = sbm.tile([128, 512], BF16, tag="m1", name="m1")
            m2 = sbm.tile([128, 512], BF16, tag="m2", name="m2")
            qA, qB = rtq[:, 0 * T:1 * T], rtq[:, 1 * T:2 * T]
            qC, qD = rtq[:, 2 * T:3 * T], rtq[:, 3 * T:4 * T]
            nc.vector.tensor_mul(m1[:], pca[:], qA[:, cs])
            nc.vector.tensor_mul(m2[:], pcb[:], qB[:, cs])
            nc.vector.tensor_sub(ra[:], m1[:], m2[:])
            nc.vector.tensor_mul(m1[:], pcb[:], qC[:, cs])
            nc.vector.tensor_mul(m2[:], pca[:], qD[:, cs])
            nc.vector.tensor_add(rb[:], m1[:], m2[:])

        # =========== phase 1: q0 pass ===========
        pq0 = proj_pass(wq0, [(0, None), (D, None)])
        # Act: psum -> sbuf bf16 copies + squares
        pcs0, sqs0 = {}, {}
        for c in range(2):
            for fi, fo in enumerate((0, D)):
                pc = sbpc.tile([128, 512], BF16, tag="pc", name="pc")
                nc.scalar.copy(pc[:], pq0[(fo, c)][:])
                pcs0[(fi, c)] = pc
        for c in range(2):
            for fi in range(2):
                sq = sbq.tile([128, 512], BF16, tag="sq", name="sq")
                nc.scalar.square(sq[:], pcs0[(fi, c)][:])
                sqs0[(fi, c)] = sq
        # DVE: rope p0
        rr0 = {}
        for c in range(2):
            cs = slice(c * 512, (c + 1) * 512)
            ra = sbra.tile([128, 512], BF16, tag="ra", name="ra")
            rb = sbra.tile([128, 512], BF16, tag="rb", name="rb")
            rope_q(pcs0[(0, c)], pcs0[(1, c)], ra, rb, cs)
            rr0[c] = (ra, rb)

        # =========== q1 pass with rstd-p0 insertions ===========
        ss0 = {c: ps_sm.tile([2, 512], F32, tag="ss", name="ss") for c in range(2)}

        def ins_ssq0():
            for c in range(2):
                for fi in range(2):
                    nc.tensor.matmul(ss0[c][:], cstb[:, 0:2], sqs0[(fi, c)][:],
                                     start=(fi == 0), stop=(fi == 1))

        rstd0 = {}
        for c in range(2):
            rstd0[c] = sbr.tile([2, 512], F32R, tag="rstd", name="rstd")

        pq1 = proj_pass(wq1, [(0, None), (D, None)],
                        inserts={10: ins_ssq0})

        # Act: sqrt ss p0 ; DVE: recip -> rstd0
        for c in range(2):
            st = sbr.tile([2, 512], F32, tag="stmp", name="stmp")
            nc.scalar.activation(st[:], ss0[c][:], SQRTF,
                                 bias=cst[0:2, 1:2], scale=float(1.0 / H))
            nc.vector.reciprocal(rstd0[c][:], st[:])
        # PE: bc p0 broadcast matmuls (after rstd0 writers are emitted)
        bps0 = {}
        for c in range(2):
            bp = ps_bps.tile([128, 512], F32, tag="bps", name="bps")
            nc.tensor.matmul(bp[:], bcw[:], rstd0[c][:], start=True, stop=True)
            bps0[c] = bp
        # Act: bps copies ; DVE: qh p0 mults
        for c in range(2):
            bsb = sbb.tile([128, 512], BF16, tag="bsb", name="bsb")
            nc.scalar.copy(bsb[:], bps0[c][:])
            cs = slice(c * 512, (c + 1) * 512)
            ra, rb = rr0[c]
            nc.vector.tensor_mul(qh[0][0:64, cs], ra[0:64, :], bsb[0:64, :])
            nc.vector.tensor_mul(qh[0][64:128, cs], rb[0:64, :], bsb[0:64, :])
            nc.vector.tensor_mul(qh[1][0:64, cs], ra[64:128, :], bsb[64:128, :])
            nc.vector.tensor_mul(qh[1][64:128, cs], rb[64:128, :], bsb[64:128, :])

        # Act: q1 copies + squares ; DVE: rope p1
        pcs1, sqs1 = {}, {}
        for c in range(2):
            for fi, fo in enumerate((0, D)):
                pc = sbpc.tile([128, 512], BF16, tag="pc", name="pc")
                nc.scalar.copy(pc[:], pq1[(fo, c)][:])
                pcs1[(fi, c)] = pc
        for c in range(2):
            for fi in range(2):
                sq = sbq.tile([128, 512], BF16, tag="sq", name="sq")
                nc.scalar.square(sq[:], pcs1[(fi, c)][:])
                sqs1[(fi, c)] = sq
        rr1 = {}
        for c in range(2):
            cs = slice(c * 512, (c + 1) * 512)
            ra = sbra.tile([128, 512], BF16, tag="ra", name="ra")
            rb = sbra.tile([128, 512], BF16, tag="rb", name="rb")
            rope_q(pcs1[(0, c)], pcs1[(1, c)], ra, rb, cs)
            rr1[c] = (ra, rb)

        # =========== k pass with ssq-p1 insertion ===========
        ss1 = {c: ps_sm.tile([2, 512], F32, tag="ss", name="ss") for c in range(2)}

        def ins_ssq1():
            for c in range(2):
                for fi in range(2):
                    nc.tensor.matmul(ss1[c][:], cstb[:, 0:2], sqs1[(fi, c)][:],
                                     start=(fi == 0), stop=(fi == 1))

        pk = proj_pass(wkv, [(0, None)], inserts={12: ins_ssq1})

        rstd1 = {c: sbr.tile([2, 512], F32R, tag="rstd", name="rstd")
                 for c in range(2)}
        for c in range(2):
            st = sbr.tile([2, 512], F32, tag="stmp", name="stmp")
            nc.scalar.activation(st[:], ss1[c][:], SQRTF,
                                 bias=cst[0:2, 1:2], scale=float(1.0 / H))
            nc.vector.reciprocal(rstd1[c][:], st[:])

        # Act: pck copies + sqk squares ; DVE: k rope
        pck = {}
        for c in range(2):
            pc = sbpc.tile([128, 512], BF16, tag="pc", name="pc")
            nc.scalar.copy(pc[:], pk[(0, c)][:])
            pck[c] = pc
            cs = slice(c * 512, (c + 1) * 512)
            nc.scalar.square(sqk[:, cs], pc[:])
        kA, kB = rtk[:, 0 * T:1 * T], rtk[:, 1 * T:2 * T]
        kC, kD = rtk[:, 2 * T:3 * T], rtk[:, 3 * T:4 * T]

        def k_rope(c):
            cs = slice(c * 512, (c + 1) * 512)
            m1 = sbm.tile([64, 512], BF16, tag="km1", name="km1")
            m2 = sbm.tile([64, 512], BF16, tag="km2", name="km2")
            k0, k1 = pck[c][0:64, :], pck[c][64:128, :]
            nc.vector.tensor_mul(m1[:], k0, kA[0:64, cs])
            nc.vector.tensor_mul(m2[:], k1, kB[64:128, cs])
            nc.vector.tensor_sub(kTn[0:64, cs], m1[:], m2[:])
            nc.vector.tensor_mul(m1[:], k1, kC[64:128, cs])
            nc.vector.tensor_mul(m2[:], k0, kD[0:64, cs])
            nc.vector.tensor_add(kTn[64:128, cs], m1[:], m2[:])

        k_rope(0)

        # =========== v pass with ksum / bc-p1 insertions ===========
        ksum = {c: ps_sm.tile([2, 512], F32, tag="ss", name="ks") for c in range(2)}
        bps1 = {}

        def ins_ksum(c):
            def f():
                nc.tensor.matmul(ksum[c][0:1, :], onesb,
                                 sqk[:, c * 512:(c + 1) * 512],
                                 start=True, stop=True)
            return f

        def ins_bc1():
            for c in range(2):
                bp = ps_bps.tile([128, 512], F32, tag="bps", name="bps")
                nc.tensor.matmul(bp[:], bcw[:], rstd1[c][:], start=True, stop=True)
                bps1[c] = bp

        units = []           # (g, tt) in half-major, g-major, tt-minor order
        for half in range(2):
            tts = [tt for tt in range(half * 4, (half + 1) * 4) if plan[tt]]
            for g in range(G):
                for tt in tts:
                    units.append((g, tt, half))
        udata = {}

        def emit_lg_exp_mask(u, lg_pool=None):
            g, tt, half = u
            ents = plan[tt]
            n_e = len(ents)
            w = n_e * 128
            lg = (lg_pool or ps_lg).tile([128, 512], F32, tag="proj" if lg_pool else "lg", name="lg")
            for ei, (si, kind, mi) in enumerate(ents):
                nc.tensor.matmul(lg[:, ei * 128:(ei + 1) * 128],
                                 kTn[:, si * 128:(si + 1) * 128],
                                 qh[g][:, tt * 128:(tt + 1) * 128],
                                 start=True, stop=True)
            P = sbP.tile([128, 512], BF16, tag="P", name="P")
            nc.scalar.activation(P[:, :w], lg[:, :w], EXPF, scale=1.0)
            spans = []
            for ei, (si, kind, mi) in enumerate(ents):
                if kind != "partial":
                    continue
                if spans and spans[-1][1] == ei:
                    spans[-1][1] = ei + 1
                else:
                    spans.append([ei, ei + 1, mi])
            for e0, e1, mi0 in spans:
                nc.vector.tensor_mul(
                    P[:, e0 * 128:e1 * 128], P[:, e0 * 128:e1 * 128],
                    mpk[:, mi0 * 128:(mi0 + (e1 - e0)) * 128])
            udata[u] = (P, n_e)

        _pre_units = [u for u in units if u[2] == 0][:5]
        _pre_done = list(_pre_units)

        # v chunk 0 first: its psums stop early so V s-tiles 0..3 are
        # transposed + copied while v chunk 1 still projects
        pv0 = {}
        for di in range(ND):
            if di == 6:
                ins_ksum(0)()
            if di == 10:
                ins_bc1()
            if di == 12:
                ins_ksum(1)()
            if di == 0:
                pv0[0] = psproj.tile([128, 512], F32, tag="proj", name="proj")
            nc.tensor.matmul(pv0[0][:], wkv[:, D + di * 128:D + (di + 1) * 128],
                             xt[:, di * T: di * T + 512],
                             start=(di == 0), stop=(di == ND - 1))
        pv = {(D, 0): pv0[0]}

        # Act: sqrt ksum (SCALE*rstd_k = 1/sqrt(ssq + H*eps))
        kst = {}
        for c in range(2):
            st = sbr.tile([1, 512], F32, tag="kst", name="kst")
            nc.scalar.activation(st[:], ksum[c][0:1, :], SQRTF,
                                 bias=cst[0:1, 0:1], scale=1.0)
            kst[c] = st
        # DVE: per-chunk recip -> Pool broadcast -> fold; chunk-0 chain runs
        # before k_rope(1) so the first attention t-tiles unblock early
        def k_fold(c):
            cs = slice(c * 512, (c + 1) * 512)
            nc.vector.reciprocal(rk[0:1, cs], kst[c][:])
            nc.gpsimd.partition_broadcast(rkb[:, cs], rk[0:1, cs])
            nc.vector.tensor_mul(kTn[:, cs], kTn[:, cs], rkb[:, cs])

        k_fold(0)
        k_rope(1)
        k_fold(1)

        # dummy exp: forces the exp act-table load into this idle window,
        # before the first real attention exp
        dmy = sbr.tile([1, 2], F32, tag="dmy", name="dmy")
        nc.scalar.activation(dmy[:], kst[1][0:1, 0:2], EXPF, scale=1.0)

        # Act: bps p1 copies ; DVE (deferred into att loop): qh p1 mults
        bsb1 = {}
        for c in range(2):
            bsb = sbb.tile([128, 512], BF16, tag="bsb", name="bsb")
            nc.scalar.copy(bsb[:], bps1[c][:])
            bsb1[c] = bsb

        def qh_p1_mults(c):
            cs = slice(c * 512, (c + 1) * 512)
            ra, rb = rr1[c]
            bsb = bsb1[c]
            nc.vector.tensor_mul(qh[2][0:64, cs], ra[0:64, :], bsb[0:64, :])
            nc.vector.tensor_mul(qh[2][64:128, cs], rb[0:64, :], bsb[0:64, :])
            nc.vector.tensor_mul(qh[3][0:64, cs], ra[64:128, :], bsb[64:128, :])
            nc.vector.tensor_mul(qh[3][64:128, cs], rb[64:128, :], bsb[64:128, :])

        # Act: vt c0 copy ; PE: transpose + V copies for s-tiles 0..3
        nc.scalar.copy(vt_sb[:, 0:512], pv[(D, 0)][:])

        def vt_group(j0, j1):
            for j in range(j0, j1):
                vp = ps_bps.tile([128, 128], BF16, tag="vtpe", name="vtpe")
                nc.tensor.transpose(vp[:], vt_sb[:, j * 128:(j + 1) * 128], iden[:])
                nc.scalar.copy(V[:, j * 128:(j + 1) * 128], vp[:])

        # v chunk 1 pass (Vt c0 transposes + early att logit units overlap)
        pv1 = psproj.tile([128, 512], F32, tag="proj", name="proj")
        for di in range(ND):
            if di == 4:
                vt_group(0, 2)
            if di == 8:
                vt_group(2, 4)
            if di >= 5 and _pre_units:
                emit_lg_exp_mask(_pre_units.pop(0), lg_pool=psproj)
            nc.tensor.matmul(pv1[:], wkv[:, D + di * 128:D + (di + 1) * 128],
                             xt[:, di * T + 512: di * T + 1024],
                             start=(di == 0), stop=(di == ND - 1))
        nc.scalar.copy(vt_sb[:, 512:1024], pv1[:])

        # =========== phase 2: attention + out projection ===========
        es1.close()   # free phase-1 SBUF + PSUM
        sbrec = pool(name="sb_rec", bufs=2)        # quad reciprocals
        sbbc = pool(name="sb_bcs", bufs=2)         # quad broadcasts
        obp = pool(name="sb_ob", bufs=2)           # output staging
        ps_lg = pool(name="ps_lg", bufs=2, space="PSUM")
        ps_vt = pool(name="ps_vt", bufs=1, space="PSUM")
        ps_qkv = pool(name="ps_qkv", bufs=2, space="PSUM")
        ps_den = pool(name="ps_den", bufs=1, space="PSUM")
        ps_op = pool(name="ps_op", bufs=2, space="PSUM")

        def vt_group2(j0, j1):
            for j in range(j0, j1):
                vp = ps_vt.tile([128, 128], BF16, tag="vtp", name="vtp")
                nc.tensor.transpose(vp[:], vt_sb[:, j * 128:(j + 1) * 128], iden[:])
                nc.scalar.copy(V[:, j * 128:(j + 1) * 128], vp[:])


        quad_ps = {}         # (g, half) -> (den_ps, qkv_ps, n_done, n_total)
        quad_cnt = {}
        for g, tt, half in units:
            quad_cnt[(g, half)] = quad_cnt.get((g, half), 0) + 1

        udata = {}


        def emit_den_qkv(u):
            g, tt, half = u
            qk = quad_ps.get((g, half))
            if qk is None:
                den = ps_den.tile([1, 512], F32, tag="den", name="den")
                qkv = ps_qkv.tile([128, 512], F32, tag="qkv", name="qkv")
                qk = quad_ps[(g, half)] = [den, qkv, 0]
            den, qkv, _ = qk
            P, n_e = udata.pop(u)
            ents = plan[tt]
            ttl = tt - half * 4
            for ei, (si, kind, mi) in enumerate(ents):
                nc.tensor.matmul(den[0:1, ttl * 128:(ttl + 1) * 128], onesb,
                                 P[:, ei * 128:(ei + 1) * 128],
                                 start=(ei == 0), stop=(ei == n_e - 1))
            for ei, (si, kind, mi) in enumerate(ents):
                nc.tensor.matmul(qkv[:, ttl * 128:(ttl + 1) * 128],
                                 V[:, si * 128:(si + 1) * 128],
                                 P[:, ei * 128:(ei + 1) * 128],
                                 start=(ei == 0), stop=(ei == n_e - 1))
            qk[2] += 1
            if qk[2] == quad_cnt[(g, half)]:
                hs = slice(half * 512, (half + 1) * 512)
                rec = sbrec.tile([1, 512], F32, tag="rec", name="rec")
                nc.vector.reciprocal(rec[:], den[0:1, :])
                bcs = sbbc.tile([128, 512], F32, tag="bcs", name="bcs")
                nc.gpsimd.partition_broadcast(bcs[:], rec[0:1, :])
                h0_ = half * 512
                nc.vector.tensor_mul(qkvh[g][:, h0_:h0_ + 256],
                                     qkv[:, 0:256], bcs[:, 0:256])
                nc.vector.tensor_mul(qkvh[g][:, h0_ + 256:h0_ + 512],
                                     qkv[:, 256:512], bcs[:, 256:512])
                del quad_ps[(g, half)]

        obs = {}

        def emit_outproj(tt, dc, w512=512):
            op = ps_op.tile([128, 512], F32, tag="op", name="op")
            for g in range(G):
                nc.tensor.matmul(op[:, 0:w512],
                                 qkvh[g][:, tt * 128:(tt + 1) * 128],
                                 wo[:, g * D + dc * 512: g * D + dc * 512 + w512],
                                 start=(g == 0), stop=(g == G - 1))
            ob = obs.get(tt)
            if ob is None:
                ob = obs[tt] = obp.tile([128, D], BF16, tag="ob", name="ob")
            if dc % 2 == 0:
                nc.vector.tensor_copy(ob[:, dc * 512:(dc + 1) * 512], op[:])
            else:
                nc.scalar.copy(ob[:, dc * 512:(dc + 1) * 512], op[:])
            if tt >= 4:
                q = (nc.sync, nc.scalar, nc.gpsimd, nc.sync)[dc]
                q.dma_start(
                    out_d[tt * 128:(tt + 1) * 128, dc * 512:(dc + 1) * 512],
                    ob[:, dc * 512:(dc + 1) * 512])
                if dc == 3:
                    del obs[tt]
            elif dc == 3:
                nc.sync.dma_start(out_d[tt * 128:(tt + 1) * 128, :], ob[:])
                del obs[tt]

        # --- attention half 0, software-pipelined depth 2 ---
        h0_units = [u for u in units if u[2] == 0]
        h1_units = [u for u in units if u[2] == 1]

        LAG = 3
        seq = []
        for i, u in enumerate(h0_units):
            if u not in _pre_done:
                seq.append(("lg", u))
            if i == 1:
                seq.append(("vt", (4, 6)))
            if i == 2:
                seq.append(("vt", (6, 8)))
                seq.append(("qhp1", 0))
            if i == len(h0_units) - 1:
                seq.append(("qhp1", 1))
            if i >= LAG:
                seq.append(("dq", h0_units[i - LAG]))
        for u in h0_units[-LAG:]:
            seq.append(("dq", u))
        # half 1 attention interleaved with half-0 out projection
        op_h0 = [(tt, dc) for tt in range(0, 4) for dc in range(4)]
        op_h1 = [(tt, dc) for tt in range(4, 8) for dc in range(4)]
        opi = 0
        for i, u in enumerate(h1_units):
            seq.append(("lg", u))
            if i >= LAG:
                seq.append(("dq", h1_units[i - LAG]))
            if i >= 2 and opi < len(op_h0):
                seq.append(("op", op_h0[opi]))
                opi += 1
        for u in h1_units[-LAG:]:
            seq.append(("dq", u))
        for rest in op_h0[opi:]:
            seq.append(("op", rest))
        for o in op_h1:
            seq.append(("op", o))

        for kind, arg in seq:
            if kind == "lg":
                emit_lg_exp_mask(arg)
            elif kind == "dq":
                emit_den_qkv(arg)
            elif kind == "vt":
                vt_group2(*arg)
            elif kind == "qhp1":
                qh_p1_mults(arg)
            elif kind == "op":
                emit_outproj(*arg)

    nc.finalize()
    return nc


_CACHE = {}


def kernel(x, segment_ids, Wq, Wk, Wv, Wo, q_scale, k_scale):
    global LAST_RESULTS
    import os
    import ml_dtypes

    bf = ml_dtypes.bfloat16
    x = np.asarray(x, np.float32)
    seg = np.asarray(segment_ids)
    Wq = np.asarray(Wq, np.float32)
    Wk = np.asarray(Wk, np.float32)
    Wv = np.asarray(Wv, np.float32)
    Wo = np.asarray(Wo, np.float32)
    q_scale = np.asarray(q_scale, np.float64)
    k_scale = np.asarray(k_scale, np.float64)

    plan, masks = _classify([seg[b] for b in range(B)])
    key = repr(plan)
    if key not in _CACHE:
        _CACHE[key] = _build_nc(plan, masks[0].shape[0])
    nc = _CACHE[key]

    half = H // 2
    timescale = ROPE_BASE ** (2.0 * np.arange(half, dtype=np.float64) / H)
    qs_lo = np.tile(q_scale[:64], 2)[:, None]
    qs_hi = np.tile(q_scale[64:], 2)[:, None]
    ks_lo = np.tile(k_scale[:64], 2)[:, None]
    ks_hi = np.tile(k_scale[64:], 2)[:, None]
    rtq_b, rtk_b = [], []
    for b in range(B):
        pos = _positions(seg[b])
        sinus = pos[None, :] / timescale[:, None]        # [64, T]
        sd = np.vstack([np.sin(sinus)] * 2)              # [128, T]
        cd = np.vstack([np.cos(sinus)] * 2)
        rtq_b.append(np.hstack([qs_lo * cd, qs_hi * sd, qs_hi * cd, qs_lo * sd]
                               ).astype(bf))
        rtk_b.append(np.hstack([ks_lo * cd, ks_hi * sd, ks_hi * cd, ks_lo * sd]
                               ).astype(bf))

    cstb = np.zeros((128, 4), np.float32)
    cstb[0:64, 0] = 1.0
    cstb[64:128, 1] = 1.0
    cstb[:, 2] = 1.0
    cst = np.zeros((128, 2), np.float32)
    cst[:, 0] = H * EPS
    cst[:, 1] = EPS
    bcw = np.zeros((2, 128), np.float32)
    bcw[0, 0:64] = 1.0
    bcw[1, 64:128] = 1.0
    iden = np.eye(128, dtype=np.float32)

    in_maps = []
    for core in range(8):
        b, kv = core // K, core % K
        xt = np.ascontiguousarray(
            x[b].T.reshape(ND, 128, T).transpose(1, 0, 2).reshape(128, ND * T))

        def qfeat(w, cols):
            # [D, 128] -> [128(d_lo), ND*128] with w[p, di*128+j] = W[di*128+p, cols[j]]
            sub = w[:, cols]                             # [D, 128]
            return sub.reshape(ND, 128, 128).transpose(1, 0, 2).reshape(128, D)

        base = kv * 4 * H
        f_cols = []
        for pair in range(2):      # (f0,f2) then (f1,f3)
            for hv in range(2):
                cols = np.concatenate([
                    np.arange(base + (2 * g4 + pair) * H + hv * 64,
                              base + (2 * g4 + pair) * H + hv * 64 + 64)
                    for g4 in range(2)])
                f_cols.append(cols)
        # heads order per pair: pair0 -> heads (0,1)?  cols above pick heads
        # (pair + 2*g4): pair0 -> heads 0,2 ... fix: want pair0 = heads 0,1.
        f_cols = []
        for pair, heads in [(0, (0, 1)), (1, (2, 3))]:
            for hv in range(2):
                cols = np.concatenate([
                    np.arange(base + g4 * H + hv * 64,
                              base + g4 * H + hv * 64 + 64) for g4 in heads])
                f_cols.append(cols)
        wq0 = np.hstack([qfeat(Wq, f_cols[0]), qfeat(Wq, f_cols[1])]).astype(bf)
        wq1 = np.hstack([qfeat(Wq, f_cols[2]), qfeat(Wq, f_cols[3])]).astype(bf)
        kcols = np.arange(kv * H, (kv + 1) * H)
        wkv = np.hstack([qfeat(Wk, kcols), qfeat(Wv, kcols)]).astype(bf)
        wo_t = np.ascontiguousarray(
            Wo[kv * 512:(kv + 1) * 512].reshape(G, 128, D)
            .transpose(1, 0, 2).reshape(128, G * D)).astype(bf)
        nm = max(masks[b].shape[0], 1)
        mpk = np.ascontiguousarray(
            masks[b].transpose(1, 0, 2).reshape(128, nm * 128)).astype(bf)

        in_maps.append({
            "xt": xt.astype(bf), "wq0": wq0, "wq1": wq1, "wkv": wkv,
            "wo": wo_t, "rtq": rtq_b[b], "rtk": rtk_b[b], "mpk": mpk,
            "iden": iden.astype(bf), "cstb": cstb.astype(bf),
            "cst": cst, "bcw": bcw,
        })

    do_trace = os.environ.get("BASS_TRACE") == "1"
    res = run_bass_kernel_spmd(
        nc, in_maps, core_ids=list(range(8)), trace=do_trace)
    LAST_RESULTS = res

    out = np.zeros((B, T, D), np.float32)
    for core in range(8):
        out[core // K] += res.results[core]["out"].astype(np.float32)
    return out


# revision 3
# speedup vs baseline: 1.0092x; 1.0092x over previous
"""Trainium2 Bass kernel v2 for segment-causal GQA attention.

Sharding: 8 cores = batch (2) x kv-head (4), as baseline. All matmul
operands bf16 (1 cyc/row at any free size), halving DMA and enabling
128-wide attention t-tiles. RMS scales folded into host rope tables;
SCALE*rstd_k folded into kTn columns so exp batches over s-tiles.
Attention: per (g, t-tile) unit = n_e logit matmuls into one psum run,
one exp, one mask mult, n_e den + n_e qkv matmuls; den/qkv batched
per (g, 4-tt quad) for one reciprocal + broadcast + normalize.
"""

import sys

sys.path.insert(0, "/opt/trn_rl_repo")

import numpy as np

import concourse.bacc as bacc
import concourse.bass as bass  # noqa: F401
import concourse.tile as tile
from concourse import mybir
from concourse.bass_utils import run_bass_kernel_spmd

B, T, D = 2, 1024, 2048
N, K, H = 16, 4, 128
G = N // K
EPS = 1e-6
ROPE_BASE = 10000.0
ND = D // 128        # 16 d-tiles
NS = T // 128        # 8 s-tiles
NT = T // 128        # 8 t-tiles
F32 = mybir.dt.float32
F32R = mybir.dt.float32r
BF16 = mybir.dt.bfloat16
EXPF = mybir.ActivationFunctionType.Exp
SQRTF = mybir.ActivationFunctionType.Sqrt

LAST_RESULTS = None  # test harness reads exec_time_ns from here


def _positions(seg):
    t = seg.shape[0]
    idx = np.arange(t, dtype=np.int64)
    is_start = np.concatenate([[True], seg[1:] != seg[:-1]])
    seg_start = np.maximum.accumulate(np.where(is_start, idx, 0))
    return (idx - seg_start).astype(np.float64)


def _classify(seg_rows):
    """Union tile classification over batches at 128x128 granularity.

    Returns (plan, masks_per_batch): plan[tt] = list of (si, kind, mask_idx);
    masks_per_batch[b] = float32 [max(n_masks,1), 128, 128] of 0/1.
    """
    idx = np.arange(T)
    valids = []
    for b in range(len(seg_rows)):
        seg = seg_rows[b]
        valids.append((seg[:, None] == seg[None, :]) & (idx[:, None] <= idx[None, :]))
    plan = []
    mask_list = [[] for _ in range(len(seg_rows))]
    n_masks = 0
    for tt in range(NT):
        t0 = tt * 128
        entries = []
        for si in range(NS):
            s0 = si * 128
            subs = [v[s0:s0 + 128, t0:t0 + 128] for v in valids]
            if not any(s.any() for s in subs):
                continue
            if all(s.all() for s in subs):
                entries.append((si, "full", -1))
            else:
                for b in range(len(seg_rows)):
                    mask_list[b].append(subs[b].astype(np.float32))
                entries.append((si, "partial", n_masks))
                n_masks += 1
        plan.append(entries)
    masks = []
    for b in range(len(seg_rows)):
        if n_masks:
            masks.append(np.ascontiguousarray(np.stack(mask_list[b]), np.float32))
        else:
            masks.append(np.zeros((1, 128, 128), np.float32))
    return plan, masks


def _build_nc(plan, n_masks):
    from contextlib import ExitStack

    nc = bacc.Bacc(None, target_bir_lowering=False, debug=False)
    MULT = mybir.AluOpType.mult

    xt_d = nc.dram_tensor("xt", [128, ND * T], BF16, kind="ExternalInput")
    wq0_d = nc.dram_tensor("wq0", [128, 2 * D], BF16, kind="ExternalInput")
    wq1_d = nc.dram_tensor("wq1", [128, 2 * D], BF16, kind="ExternalInput")
    wkv_d = nc.dram_tensor("wkv", [128, 2 * D], BF16, kind="ExternalInput")
    wo_d = nc.dram_tensor("wo", [128, G * D], BF16, kind="ExternalInput")
    rtq_d = nc.dram_tensor("rtq", [128, 4 * T], BF16, kind="ExternalInput")
    rtk_d = nc.dram_tensor("rtk", [128, 4 * T], BF16, kind="ExternalInput")
    nm = max(n_masks, 1)
    mpk_d = nc.dram_tensor("mpk", [128, nm * 128], BF16, kind="ExternalInput")
    iden_d = nc.dram_tensor("iden", [128, 128], BF16, kind="ExternalInput")
    cstb_d = nc.dram_tensor("cstb", [128, 4], BF16, kind="ExternalInput")
    cst_d = nc.dram_tensor("cst", [128, 2], F32, kind="ExternalInput")
    bc_d = nc.dram_tensor("bcw", [2, 128], F32R, kind="ExternalInput")
    out_d = nc.dram_tensor("out", [T, D], BF16, kind="ExternalOutput")

    es = ExitStack()
    with es:
        es.enter_context(nc.allow_low_precision("bf16 kernel"))
        tc = es.enter_context(tile.TileContext(nc))
        pool = lambda *a, **k: es.enter_context(tc.tile_pool(*a, **k))
        pp = pool(name="persist", bufs=1)

        # ---------------- persistent SBUF tiles ----------------
        xt = pp.tile([128, ND * T], BF16, tag="xt", name="xt")
        wq0 = pp.tile([128, 2 * D], BF16, tag="wq0", name="wq0")
        wq1 = pp.tile([128, 2 * D], BF16, tag="wq1", name="wq1")
        wkv = pp.tile([128, 2 * D], BF16, tag="wkv", name="wkv")
        wo = pp.tile([128, G * D], BF16, tag="wo", name="wo")
        rtq = pp.tile([128, 4 * T], BF16, tag="rtq", name="rtq")
        rtk = pp.tile([128, 4 * T], BF16, tag="rtk", name="rtk")
        mpk = pp.tile([128, nm * 128], BF16, tag="mpk", name="mpk")
        iden = pp.tile([128, 128], BF16, tag="iden", name="iden")
        cstb = pp.tile([128, 4], BF16, tag="cstb", name="cstb")
        cst = pp.tile([128, 2], F32, tag="cst", name="cst")
        bcw = pp.tile([2, 128], F32R, tag="bcw", name="bcw")
        qh = [pp.tile([128, T], BF16, tag=f"qh{g}", name=f"qh{g}") for g in range(G)]
        kTn = pp.tile([128, T], BF16, tag="kTn", name="kTn")
        V = pp.tile([128, T], BF16, tag="V", name="V")
        vt_sb = pp.tile([128, T], BF16, tag="vt", name="vt")
        sqk = pp.tile([128, T], BF16, tag="sqk", name="sqk")
        qkvh = [pp.tile([128, T], BF16, tag=f"qkvh{g}", name=f"qkvh{g}")
                for g in range(G)]
        rk = pp.tile([1, T], F32, tag="rk", name="rk")
        rkb = pp.tile([128, T], F32, tag="rkb", name="rkb")

        onesb = cstb[:, 2:3]          # bf16 ones column (den / ksum lhsT)

        # ---------------- DMAs (sync queue, priority order) ----------------
        def xchunk(ci):
            sl = slice(ci * 2 * T, (ci + 1) * 2 * T)
            nc.sync.dma_start(xt[:, sl], xt_d[:, sl])

        nc.sync.dma_start(xt[:, 0:T], xt_d[:, 0:T])                # di0
        nc.sync.dma_start(wq0[:, 0:512], wq0_d[:, 0:512])          # f0 di0-3
        nc.sync.dma_start(xt[:, T:2 * T], xt_d[:, T:2 * T])        # di1
        nc.sync.dma_start(wq0[:, D:D + 512], wq0_d[:, D:D + 512])  # f2 di0-3
        xchunk(1)
        nc.sync.dma_start(wq0[:, 512:D], wq0_d[:, 512:D])          # f0 di4-15
        xchunk(2)
        nc.sync.dma_start(wq0[:, D + 512:2 * D], wq0_d[:, D + 512:2 * D])
        xchunk(3)
        for t_, d_ in [(cstb, cstb_d), (cst, cst_d), (bcw, bc_d),
                       (iden, iden_d)]:
            nc.sync.dma_start(t_[:], d_[:])
        for ci in range(4, 8):
            xchunk(ci)
        nc.sync.dma_start(wq1[:], wq1_d[:])
        nc.sync.dma_start(rtq[:], rtq_d[:])
        nc.sync.dma_start(wkv[:], wkv_d[:])
        nc.sync.dma_start(rtk[:], rtk_d[:])
        nc.sync.dma_start(wo[:], wo_d[:])
        nc.sync.dma_start(mpk[:], mpk_d[:])

        # ---------------- stream pools ----------------
        sbra = pool(name="sb_ra", bufs=4)          # rope outputs ra/rb
        sbb = pool(name="sb_bps", bufs=2)          # bps broadcast sbuf
        sbP = pool(name="sb_P", bufs=8)            # attention P tiles
        es1 = ExitStack()
        pool1 = lambda *a, **k: es1.enter_context(tc.tile_pool(*a, **k))
        sbpc = pool1(name="sb_pc", bufs=6)         # psum->sbuf proj copies
        sbm = pool1(name="sb_m", bufs=2)           # rope temporaries
        sbq = pool1(name="sb_sq", bufs=4)          # square tiles
        sbr = pool1(name="sb_rstd", bufs=2)        # rstd / stmp tiles

        psproj = pool1(name="ps_proj", bufs=4, space="PSUM")
        ps_sm = pool1(name="ps_small", bufs=2, space="PSUM")
        ps_bps = pool1(name="ps_bps", bufs=1, space="PSUM")

        # PE p-state warmup: matmuls on an uninitialized scratch tile (values
        # never read) while the input DMAs stream in
        wsc = sbm.tile([128, 128], BF16, tag="m1", name="wscratch")
        nc.gpsimd.memset(wsc[:], 0.0)
        warm = psproj.tile([128, 512], F32, tag="proj", name="warm")
        for _ in range(18):
            nc.tensor.matmul(warm[:, 0:128], wsc[:], wsc[:],
                             start=True, stop=True)
        wdmy = sbr.tile([1, 2], F32, tag="dmy", name="wdmy")
        nc.scalar.copy(wdmy[:], warm[0:1, 0:2])

        def proj_pass(w, feats, inserts=None, fillers=0, pre_pss=None):
            """d-outer pass over `feats` = list of (col_off, psum pair).
            inserts: {di: fn} PE-stream injections. fillers: p-state keepalive
            matmuls per di boundary while the xt stream is still arriving."""
            pss = dict(pre_pss) if pre_pss else {}
            for fo, _ in feats:
                for c in range(2):
                    if (fo, c) not in pss:
                        pss[(fo, c)] = psproj.tile([128, 512], F32, tag="proj",
                                                   name="proj")
            for di in range(ND):
                if inserts and di in inserts:
                    inserts[di]()
                if fillers and di < 8:
                    for _ in range(fillers):
                        nc.tensor.matmul(warm[:, 128:256], wsc[:], wsc[:],
                                         start=True, stop=True)
                for fo, _ in feats:
                    for c in range(2):
                        nc.tensor.matmul(
                            pss[(fo, c)][:],
                            w[:, fo + di * 128: fo + (di + 1) * 128],
                            xt[:, di * T + c * 512: di * T + (c + 1) * 512],
                            start=(di == 0), stop=(di == ND - 1))
            return pss

        def rope_q(pca, pcb, ra, rb, cs):
            m1 = sbm.tile([128, 512], BF16, tag="m1", name="m1")
            m2 = sbm.tile([128, 512], BF16, tag="m2", name="m2")
            qA, qB = rtq[:, 0 * T:1 * T], rtq[:, 1 * T:2 * T]
            qC, qD = rtq[:, 2 * T:3 * T], rtq[:, 3 * T:4 * T]
            nc.vector.tensor_mul(m1[:], pca[:], qA[:, cs])
            nc.vector.tensor_mul(m2[:], pcb[:], qB[:, cs])
            nc.vector.tensor_sub(ra[:], m1[:], m2[:])
            nc.vector.tensor_mul(m1[:], pcb[:], qC[:, cs])
            nc.vector.tensor_mul(m2[:], pca[:], qD[:, cs])
            nc.vector.tensor_add(rb[:], m1[:], m2[:])

        # =========== phase 1: q0 pass ===========
        pq0 = proj_pass(wq0, [(0, None), (D, None)])
        # Act: psum -> sbuf bf16 copies + squares
        pcs0, sqs0 = {}, {}
        for c in range(2):
            for fi, fo in enumerate((0, D)):
                pc = sbpc.tile([128, 512], BF16, tag="pc", name="pc")
                nc.scalar.copy(pc[:], pq0[(fo, c)][:])
                pcs0[(fi, c)] = pc
        for c in range(2):
            for fi in range(2):
                sq = sbq.tile([128, 512], BF16, tag="sq", name="sq")
                nc.scalar.square(sq[:], pcs0[(fi, c)][:])
                sqs0[(fi, c)] = sq
        # DVE: rope p0
        rr0 = {}
        for c in range(2):
            cs = slice(c * 512, (c + 1) * 512)
            ra = sbra.tile([128, 512], BF16, tag="ra", name="ra")
            rb = sbra.tile([128, 512], BF16, tag="rb", name="rb")
            rope_q(pcs0[(0, c)], pcs0[(1, c)], ra, rb, cs)
            rr0[c] = (ra, rb)

        # =========== q1 pass with rstd-p0 insertions ===========
        ss0 = {c: ps_sm.tile([2, 512], F32, tag="ss", name="ss") for c in range(2)}

        def ins_ssq0():
            for c in range(2):
                for fi in range(2):
                    nc.tensor.matmul(ss0[c][:], cstb[:, 0:2], sqs0[(fi, c)][:],
                                     start=(fi == 0), stop=(fi == 1))

        rstd0 = {}
        for c in range(2):
            rstd0[c] = sbr.tile([2, 512], F32R, tag="rstd", name="rstd")

        pq1 = proj_pass(wq1, [(0, None), (D, None)],
                        inserts={10: ins_ssq0},
                        pre_pss={(0, 0): ps_bps.tile([128, 512], F32,
                                                     tag="bps", name="q1pre")})

        # Act: sqrt ss p0 ; DVE: recip -> rstd0
        for c in range(2):
            st = sbr.tile([2, 512], F32, tag="stmp", name="stmp")
            nc.scalar.activation(st[:], ss0[c][:], SQRTF,
                                 bias=cst[0:2, 1:2], scale=float(1.0 / H))
            nc.vector.reciprocal(rstd0[c][:], st[:])
        # PE: bc p0 broadcast matmuls (after rstd0 writers are emitted)
        bps0 = {}
        for c in range(2):
            bp = ps_bps.tile([128, 512], F32, tag="bps", name="bps")
            nc.tensor.matmul(bp[:], bcw[:], rstd0[c][:], start=True, stop=True)
            bps0[c] = bp
        # Act: bps copies ; DVE: qh p0 mults
        for c in range(2):
            bsb = sbb.tile([128, 512], BF16, tag="bsb", name="bsb")
            nc.scalar.copy(bsb[:], bps0[c][:])
            cs = slice(c * 512, (c + 1) * 512)
            ra, rb = rr0[c]
            nc.vector.tensor_mul(qh[0][0:64, cs], ra[0:64, :], bsb[0:64, :])
            nc.vector.tensor_mul(qh[0][64:128, cs], rb[0:64, :], bsb[0:64, :])
            nc.vector.tensor_mul(qh[1][0:64, cs], ra[64:128, :], bsb[64:128, :])
            nc.vector.tensor_mul(qh[1][64:128, cs], rb[64:128, :], bsb[64:128, :])

        # Act: q1 copies + squares ; DVE: rope p1
        pcs1, sqs1 = {}, {}
        for c in range(2):
            for fi, fo in enumerate((0, D)):
                pc = sbpc.tile([128, 512], BF16, tag="pc", name="pc")
                nc.scalar.copy(pc[:], pq1[(fo, c)][:])
                pcs1[(fi, c)] = pc
        for c in range(2):
            for fi in range(2):
                sq = sbq.tile([128, 512], BF16, tag="sq", name="sq")
                nc.scalar.square(sq[:], pcs1[(fi, c)][:])
                sqs1[(fi, c)] = sq
        rr1 = {}
        for c in range(2):
            cs = slice(c * 512, (c + 1) * 512)
            ra = sbra.tile([128, 512], BF16, tag="ra", name="ra")
            rb = sbra.tile([128, 512], BF16, tag="rb", name="rb")
            rope_q(pcs1[(0, c)], pcs1[(1, c)], ra, rb, cs)
            rr1[c] = (ra, rb)

        # =========== k pass with ssq-p1 insertion ===========
        ss1 = {c: ps_sm.tile([2, 512], F32, tag="ss", name="ss") for c in range(2)}

        def ins_ssq1():
            for c in range(2):
                for fi in range(2):
                    nc.tensor.matmul(ss1[c][:], cstb[:, 0:2], sqs1[(fi, c)][:],
                                     start=(fi == 0), stop=(fi == 1))

        pk = proj_pass(wkv, [(0, None)], inserts={12: ins_ssq1})

        rstd1 = {c: sbr.tile([2, 512], F32R, tag="rstd", name="rstd")
                 for c in range(2)}
        for c in range(2):
            st = sbr.tile([2, 512], F32, tag="stmp", name="stmp")
            nc.scalar.activation(st[:], ss1[c][:], SQRTF,
                                 bias=cst[0:2, 1:2], scale=float(1.0 / H))
            nc.vector.reciprocal(rstd1[c][:], st[:])

        # Act: pck copies + sqk squares ; DVE: k rope
        pck = {}
        for c in range(2):
            pc = sbpc.tile([128, 512], BF16, tag="pc", name="pc")
            nc.scalar.copy(pc[:], pk[(0, c)][:])
            pck[c] = pc
            cs = slice(c * 512, (c + 1) * 512)
            nc.scalar.square(sqk[:, cs], pc[:])
        kA, kB = rtk[:, 0 * T:1 * T], rtk[:, 1 * T:2 * T]
        kC, kD = rtk[:, 2 * T:3 * T], rtk[:, 3 * T:4 * T]

        def k_rope(c):
            cs = slice(c * 512, (c + 1) * 512)
            m1 = sbm.tile([64, 512], BF16, tag="km1", name="km1")
            m2 = sbm.tile([64, 512], BF16, tag="km2", name="km2")
            k0, k1 = pck[c][0:64, :], pck[c][64:128, :]
            nc.vector.tensor_mul(m1[:], k0, kA[0:64, cs])
            nc.vector.tensor_mul(m2[:], k1, kB[64:128, cs])
            nc.vector.tensor_sub(kTn[0:64, cs], m1[:], m2[:])
            nc.vector.tensor_mul(m1[:], k1, kC[64:128, cs])
            nc.vector.tensor_mul(m2[:], k0, kD[0:64, cs])
            nc.vector.tensor_add(kTn[64:128, cs], m1[:], m2[:])

        k_rope(0)

        # =========== v pass with ksum / bc-p1 insertions ===========
        ksum = {c: ps_sm.tile([2, 512], F32, tag="ss", name="ks") for c in range(2)}
        bps1 = {}

        def ins_ksum(c):
            def f():
                nc.tensor.matmul(ksum[c][0:1, :], onesb,
                                 sqk[:, c * 512:(c + 1) * 512],
                                 start=True, stop=True)
            return f

        def ins_bc1():
            for c in range(2):
                bp = ps_bps.tile([128, 512], F32, tag="bps", name="bps")
                nc.tensor.matmul(bp[:], bcw[:], rstd1[c][:], start=True, stop=True)
                bps1[c] = bp

        units = []           # (g, tt) in half-major, g-major, tt-minor order
        for half in range(2):
            tts = [tt for tt in range(half * 4, (half + 1) * 4) if plan[tt]]
            for g in range(G):
                for tt in tts:
                    units.append((g, tt, half))
        udata = {}

        def emit_lg_exp_mask(u, lg_pool=None):
            g, tt, half = u
            ents = plan[tt]
            n_e = len(ents)
            w = n_e * 128
            lg = (lg_pool or ps_lg).tile([128, 512], F32, tag="proj" if lg_pool else "lg", name="lg")
            for ei, (si, kind, mi) in enumerate(ents):
                nc.tensor.matmul(lg[:, ei * 128:(ei + 1) * 128],
                                 kTn[:, si * 128:(si + 1) * 128],
                                 qh[g][:, tt * 128:(tt + 1) * 128],
                                 start=True, stop=True)
            P = sbP.tile([128, 512], BF16, tag="P", name="P")
            nc.scalar.activation(P[:, :w], lg[:, :w], EXPF, scale=1.0)
            spans = []
            for ei, (si, kind, mi) in enumerate(ents):
                if kind != "partial":
                    continue
                if spans and spans[-1][1] == ei:
                    spans[-1][1] = ei + 1
                else:
                    spans.append([ei, ei + 1, mi])
            for e0, e1, mi0 in spans:
                nc.vector.tensor_mul(
                    P[:, e0 * 128:e1 * 128], P[:, e0 * 128:e1 * 128],
                    mpk[:, mi0 * 128:(mi0 + (e1 - e0)) * 128])
            udata[u] = (P, n_e)

        _pre_units = [u for u in units if u[2] == 0][:4]
        _pre_done = list(_pre_units)

        # v chunk 0 first: its psums stop early so V s-tiles 0..3 are
        # transposed + copied while v chunk 1 still projects
        pv0 = {}
        for di in range(ND):
            if di == 6:
                ins_ksum(0)()
            if di == 10:
                ins_bc1()
            if di == 12:
                ins_ksum(1)()
            if di == 0:
                pv0[0] = psproj.tile([128, 512], F32, tag="proj", name="proj")
            nc.tensor.matmul(pv0[0][:], wkv[:, D + di * 128:D + (di + 1) * 128],
                             xt[:, di * T: di * T + 512],
                             start=(di == 0), stop=(di == ND - 1))
        pv = {(D, 0): pv0[0]}

        # Act: sqrt ksum (SCALE*rstd_k = 1/sqrt(ssq + H*eps))
        kst = {}
        for c in range(2):
            st = sbr.tile([1, 512], F32, tag="kst", name="kst")
            nc.scalar.activation(st[:], ksum[c][0:1, :], SQRTF,
                                 bias=cst[0:1, 0:1], scale=1.0)
            kst[c] = st
        # DVE: per-chunk recip -> Pool broadcast -> fold; chunk-0 chain runs
        # before k_rope(1) so the first attention t-tiles unblock early
        def k_fold(c):
            cs = slice(c * 512, (c + 1) * 512)
            nc.vector.reciprocal(rk[0:1, cs], kst[c][:])
            nc.gpsimd.partition_broadcast(rkb[:, cs], rk[0:1, cs])
            nc.vector.tensor_mul(kTn[:, cs], kTn[:, cs], rkb[:, cs])

        k_fold(0)
        k_rope(1)
        k_fold(1)

        # dummy exp: forces the exp act-table load into this idle window,
        # before the first real attention exp
        dmy = sbr.tile([1, 2], F32, tag="dmy", name="dmy")
        nc.scalar.activation(dmy[:], kst[1][0:1, 0:2], EXPF, scale=1.0)

        # Act: bps p1 copies ; DVE (deferred into att loop): qh p1 mults
        bsb1 = {}
        for c in range(2):
            bsb = sbb.tile([128, 512], BF16, tag="bsb", name="bsb")
            nc.scalar.copy(bsb[:], bps1[c][:])
            bsb1[c] = bsb

        def qh_p1_mults(c):
            cs = slice(c * 512, (c + 1) * 512)
            ra, rb = rr1[c]
            bsb = bsb1[c]
            nc.vector.tensor_mul(qh[2][0:64, cs], ra[0:64, :], bsb[0:64, :])
            nc.vector.tensor_mul(qh[2][64:128, cs], rb[0:64, :], bsb[0:64, :])
            nc.vector.tensor_mul(qh[3][0:64, cs], ra[64:128, :], bsb[64:128, :])
            nc.vector.tensor_mul(qh[3][64:128, cs], rb[64:128, :], bsb[64:128, :])

        # Act: vt c0 copy ; PE: transpose + V copies for s-tiles 0..3
        nc.scalar.copy(vt_sb[:, 0:512], pv[(D, 0)][:])

        def vt_group(j0, j1):
            for j in range(j0, j1):
                vp = ps_bps.tile([128, 128], BF16, tag="vtpe", name="vtpe")
                nc.tensor.transpose(vp[:], vt_sb[:, j * 128:(j + 1) * 128], iden[:])
                nc.scalar.copy(V[:, j * 128:(j + 1) * 128], vp[:])

        # v chunk 1 pass (Vt c0 transposes + early att logit units overlap)
        pv1 = psproj.tile([128, 512], F32, tag="proj", name="proj")
        for di in range(ND):
            if di == 4:
                vt_group(0, 2)
            if di == 8:
                vt_group(2, 4)
            if di >= 5 and _pre_units:
                emit_lg_exp_mask(_pre_units.pop(0), lg_pool=psproj)
            nc.tensor.matmul(pv1[:], wkv[:, D + di * 128:D + (di + 1) * 128],
                             xt[:, di * T + 512: di * T + 1024],
                             start=(di == 0), stop=(di == ND - 1))
        nc.scalar.copy(vt_sb[:, 512:1024], pv1[:])

        # =========== phase 2: attention + out projection ===========
        es1.close()   # free phase-1 SBUF + PSUM
        sbrec = pool(name="sb_rec", bufs=2)        # quad reciprocals
        sbbc = pool(name="sb_bcs", bufs=2)         # quad broadcasts
        obp = pool(name="sb_ob", bufs=3)           # output staging
        ps_lg = pool(name="ps_lg", bufs=2, space="PSUM")
        ps_qkv = pool(name="ps_qkv", bufs=2, space="PSUM")
        ps_den = pool(name="ps_den", bufs=1, space="PSUM")
        ps_op = pool(name="ps_op", bufs=3, space="PSUM")

        def vt_group2(j0, j1):
            for j in range(j0, j1):
                vp = ps_op.tile([128, 128], BF16, tag="op", name="vtp")
                nc.tensor.transpose(vp[:], vt_sb[:, j * 128:(j + 1) * 128], iden[:])
                nc.scalar.copy(V[:, j * 128:(j + 1) * 128], vp[:])


        quad_ps = {}         # (g, half) -> (den_ps, qkv_ps, n_done, n_total)
        quad_cnt = {}
        for g, tt, half in units:
            quad_cnt[(g, half)] = quad_cnt.get((g, half), 0) + 1

        udata = {}


        def emit_den_qkv(u):
            g, tt, half = u
            qk = quad_ps.get((g, half))
            if qk is None:
                den = ps_den.tile([1, 512], F32, tag="den", name="den")
                qkv = ps_qkv.tile([128, 512], F32, tag="qkv", name="qkv")
                qk = quad_ps[(g, half)] = [den, qkv, 0]
            den, qkv, _ = qk
            P, n_e = udata.pop(u)
            ents = plan[tt]
            ttl = tt - half * 4
            for ei, (si, kind, mi) in enumerate(ents):
                nc.tensor.matmul(den[0:1, ttl * 128:(ttl + 1) * 128], onesb,
                                 P[:, ei * 128:(ei + 1) * 128],
                                 start=(ei == 0), stop=(ei == n_e - 1))
            for ei, (si, kind, mi) in enumerate(ents):
                nc.tensor.matmul(qkv[:, ttl * 128:(ttl + 1) * 128],
                                 V[:, si * 128:(si + 1) * 128],
                                 P[:, ei * 128:(ei + 1) * 128],
                                 start=(ei == 0), stop=(ei == n_e - 1))
            qk[2] += 1
            if qk[2] == quad_cnt[(g, half)]:
                hs = slice(half * 512, (half + 1) * 512)
                rec = sbrec.tile([1, 512], F32, tag="rec", name="rec")
                nc.vector.reciprocal(rec[:], den[0:1, :])
                bcs = sbbc.tile([128, 512], F32, tag="bcs", name="bcs")
                nc.gpsimd.partition_broadcast(bcs[:], rec[0:1, :])
                h0_ = half * 512
                nc.vector.tensor_mul(qkvh[g][:, h0_:h0_ + 256],
                                     qkv[:, 0:256], bcs[:, 0:256])
                nc.vector.tensor_mul(qkvh[g][:, h0_ + 256:h0_ + 512],
                                     qkv[:, 256:512], bcs[:, 256:512])
                del quad_ps[(g, half)]

        obs = {}

        def emit_outproj(tt, dc, w512=512):
            op = ps_op.tile([128, 512], F32, tag="op", name="op")
            for g in range(G):
                nc.tensor.matmul(op[:, 0:w512],
                                 qkvh[g][:, tt * 128:(tt + 1) * 128],
                                 wo[:, g * D + dc * 512: g * D + dc * 512 + w512],
                                 start=(g == 0), stop=(g == G - 1))
            ob = obs.get(tt)
            if ob is None:
                ob = obs[tt] = obp.tile([128, D], BF16, tag="ob", name="ob")
            if dc % 2 == 0:
                nc.vector.tensor_copy(ob[:, dc * 512:(dc + 1) * 512], op[:])
            else:
                nc.scalar.copy(ob[:, dc * 512:(dc + 1) * 512], op[:])
            if tt >= 4:
                q = (nc.sync, nc.scalar, nc.gpsimd, nc.sync)[dc]
                q.dma_start(
                    out_d[tt * 128:(tt + 1) * 128, dc * 512:(dc + 1) * 512],
                    ob[:, dc * 512:(dc + 1) * 512])
                if dc == 3:
                    del obs[tt]
            elif dc == 3:
                nc.sync.dma_start(out_d[tt * 128:(tt + 1) * 128, :], ob[:])
                del obs[tt]

        # --- attention half 0, software-pipelined depth 2 ---
        h0_units = [u for u in units if u[2] == 0]
        h1_units = [u for u in units if u[2] == 1]

        LAG = 3
        seq = []
        for i, u in enumerate(h0_units):
            if u not in _pre_done:
                seq.append(("lg", u))
            if i == 1:
                seq.append(("vt", (4, 6)))
            if i == 2:
                seq.append(("vt", (6, 8)))
                seq.append(("qhp1", 0))
            if i == len(h0_units) - 1:
                seq.append(("qhp1", 1))
            if i >= LAG:
                seq.append(("dq", h0_units[i - LAG]))
        for u in h0_units[-LAG:]:
            seq.append(("dq", u))
        # half 1 attention interleaved with half-0 out projection
        op_h0 = [(tt, dc) for tt in range(0, 4) for dc in range(4)]
        op_h1 = [(tt, dc) for tt in range(4, 8) for dc in range(4)]
        opi = 0
        for i, u in enumerate(h1_units):
            seq.append(("lg", u))
            if i >= LAG:
                seq.append(("dq", h1_units[i - LAG]))
            if i >= 2 and opi < len(op_h0):
                seq.append(("op", op_h0[opi]))
                opi += 1
        for u in h1_units[-LAG:]:
            seq.append(("dq", u))
        for rest in op_h0[opi:]:
            seq.append(("op", rest))
        for o in op_h1:
            seq.append(("op", o))

        for kind, arg in seq:
            if kind == "lg":
                emit_lg_exp_mask(arg)
            elif kind == "dq":
                emit_den_qkv(arg)
            elif kind == "vt":
                vt_group2(*arg)
            elif kind == "qhp1":
                qh_p1_mults(arg)
            elif kind == "op":
                emit_outproj(*arg)

    nc.finalize()
    return nc


_CACHE = {}


def kernel(x, segment_ids, Wq, Wk, Wv, Wo, q_scale, k_scale):
    global LAST_RESULTS
    import os
    import ml_dtypes

    bf = ml_dtypes.bfloat16
    x = np.asarray(x, np.float32)
    seg = np.asarray(segment_ids)
    Wq = np.asarray(Wq, np.float32)
    Wk = np.asarray(Wk, np.float32)
    Wv = np.asarray(Wv, np.float32)
    Wo = np.asarray(Wo, np.float32)
    q_scale = np.asarray(q_scale, np.float64)
    k_scale = np.asarray(k_scale, np.float64)

    plan, masks = _classify([seg[b] for b in range(B)])
    key = repr(plan)
    if key not in _CACHE:
        _CACHE[key] = _build_nc(plan, masks[0].shape[0])
    nc = _CACHE[key]

    half = H // 2
    timescale = ROPE_BASE ** (2.0 * np.arange(half, dtype=np.float64) / H)
    qs_lo = np.tile(q_scale[:64], 2)[:, None]
    qs_hi = np.tile(q_scale[64:], 2)[:, None]
    ks_lo = np.tile(k_scale[:64], 2)[:, None]
    ks_hi = np.tile(k_scale[64:], 2)[:, None]
    rtq_b, rtk_b = [], []
    for b in range(B):
        pos = _positions(seg[b])
        sinus = pos[None, :] / timescale[:, None]        # [64, T]
        sd = np.vstack([np.sin(sinus)] * 2)              # [128, T]
        cd = np.vstack([np.cos(sinus)] * 2)
        rtq_b.append(np.hstack([qs_lo * cd, qs_hi * sd, qs_hi * cd, qs_lo * sd]
                               ).astype(bf))
        rtk_b.append(np.hstack([ks_lo * cd, ks_hi * sd, ks_hi * cd, ks_lo * sd]
                               ).astype(bf))

    cstb = np.zeros((128, 4), np.float32)
    cstb[0:64, 0] = 1.0
    cstb[64:128, 1] = 1.0
    cstb[:, 2] = 1.0
    cst = np.zeros((128, 2), np.float32)
    cst[:, 0] = H * EPS
    cst[:, 1] = EPS
    bcw = np.zeros((2, 128), np.float32)
    bcw[0, 0:64] = 1.0
    bcw[1, 64:128] = 1.0
    iden = np.eye(128, dtype=np.float32)

    in_maps = []
    for core in range(8):
        b, kv = core // K, core % K
        xt = np.ascontiguousarray(
            x[b].T.reshape(ND, 128, T).transpose(1, 0, 2).reshape(128, ND * T))

        def qfeat(w, cols):
            # [D, 128] -> [128(d_lo), ND*128] with w[p, di*128+j] = W[di*128+p, cols[j]]
            sub = w[:, cols]                             # [D, 128]
            return sub.reshape(ND, 128, 128).transpose(1, 0, 2).reshape(128, D)

        base = kv * 4 * H
        f_cols = []
        for pair in range(2):      # (f0,f2) then (f1,f3)
            for hv in range(2):
                cols = np.concatenate([
                    np.arange(base + (2 * g4 + pair) * H + hv * 64,
                              base + (2 * g4 + pair) * H + hv * 64 + 64)
                    for g4 in range(2)])
                f_cols.append(cols)
        # heads order per pair: pair0 -> heads (0,1)?  cols above pick heads
        # (pair + 2*g4): pair0 -> heads 0,2 ... fix: want pair0 = heads 0,1.
        f_cols = []
        for pair, heads in [(0, (0, 1)), (1, (2, 3))]:
            for hv in range(2):
                cols = np.concatenate([
                    np.arange(base + g4 * H + hv * 64,
                              base + g4 * H + hv * 64 + 64) for g4 in heads])
                f_cols.append(cols)
        wq0 = np.hstack([qfeat(Wq, f_cols[0]), qfeat(Wq, f_cols[1])]).astype(bf)
        wq1 = np.hstack([qfeat(Wq, f_cols[2]), qfeat(Wq, f_cols[3])]).astype(bf)
        kcols = np.arange(kv * H, (kv + 1) * H)
        wkv = np.hstack([qfeat(Wk, kcols), qfeat(Wv, kcols)]).astype(bf)
        wo_t = np.ascontiguousarray(
            Wo[kv * 512:(kv + 1) * 512].reshape(G, 128, D)
            .transpose(1, 0, 2).reshape(128, G * D)).astype(bf)
        nm = max(masks[b].shape[0], 1)
        mpk = np.ascontiguousarray(
            masks[b].transpose(1, 0, 2).reshape(128, nm * 128)).astype(bf)

        in_maps.append({
            "xt": xt.astype(bf), "wq0": wq0, "wq1": wq1, "wkv": wkv,
            "wo": wo_t, "rtq": rtq_b[b], "rtk": rtk_b[b], "mpk": mpk,
            "iden": iden.astype(bf), "cstb": cstb.astype(bf),
            "cst": cst, "bcw": bcw,
        })

    do_trace = os.environ.get("BASS_TRACE") == "1"
    res = run_bass_kernel_spmd(
        nc, in_maps, core_ids=list(range(8)), trace=do_trace)
    LAST_RESULTS = res

    out = np.zeros((B, T, D), np.float32)
    for core in range(8):
        out[core // K] += res.results[core]["out"].astype(np.float32)
    return out


# revision 4
# speedup vs baseline: 1.0116x; 1.0023x over previous
"""Trainium2 Bass kernel v2 for segment-causal GQA attention.

Sharding: 8 cores = batch (2) x kv-head (4), as baseline. All matmul
operands bf16 (1 cyc/row at any free size), halving DMA and enabling
128-wide attention t-tiles. RMS scales folded into host rope tables;
SCALE*rstd_k folded into kTn columns so exp batches over s-tiles.
Attention: per (g, t-tile) unit = n_e logit matmuls into one psum run,
one exp, one mask mult, n_e den + n_e qkv matmuls; den/qkv batched
per (g, 4-tt quad) for one reciprocal + broadcast + normalize.
"""

import sys

sys.path.insert(0, "/opt/trn_rl_repo")

import numpy as np

import concourse.bacc as bacc
import concourse.bass as bass  # noqa: F401
import concourse.tile as tile
from concourse import mybir
from concourse.bass_utils import run_bass_kernel_spmd

B, T, D = 2, 1024, 2048
N, K, H = 16, 4, 128
G = N // K
EPS = 1e-6
ROPE_BASE = 10000.0
ND = D // 128        # 16 d-tiles
NS = T // 128        # 8 s-tiles
NT = T // 128        # 8 t-tiles
F32 = mybir.dt.float32
F32R = mybir.dt.float32r
BF16 = mybir.dt.bfloat16
EXPF = mybir.ActivationFunctionType.Exp
SQRTF = mybir.ActivationFunctionType.Sqrt

LAST_RESULTS = None  # test harness reads exec_time_ns from here


def _positions(seg):
    t = seg.shape[0]
    idx = np.arange(t, dtype=np.int64)
    is_start = np.concatenate([[True], seg[1:] != seg[:-1]])
    seg_start = np.maximum.accumulate(np.where(is_start, idx, 0))
    return (idx - seg_start).astype(np.float64)


def _classify(seg_rows):
    """Union tile classification over batches at 128x128 granularity.

    Returns (plan, masks_per_batch): plan[tt] = list of (si, kind, mask_idx);
    masks_per_batch[b] = float32 [max(n_masks,1), 128, 128] of 0/1.
    """
    idx = np.arange(T)
    valids = []
    for b in range(len(seg_rows)):
        seg = seg_rows[b]
        valids.append((seg[:, None] == seg[None, :]) & (idx[:, None] <= idx[None, :]))
    plan = []
    mask_list = [[] for _ in range(len(seg_rows))]
    n_masks = 0
    for tt in range(NT):
        t0 = tt * 128
        entries = []
        for si in range(NS):
            s0 = si * 128
            subs = [v[s0:s0 + 128, t0:t0 + 128] for v in valids]
            if not any(s.any() for s in subs):
                continue
            if all(s.all() for s in subs):
                entries.append((si, "full", -1))
            else:
                for b in range(len(seg_rows)):
                    mask_list[b].append(subs[b].astype(np.float32))
                entries.append((si, "partial", n_masks))
                n_masks += 1
        plan.append(entries)
    masks = []
    for b in range(len(seg_rows)):
        if n_masks:
            masks.append(np.ascontiguousarray(np.stack(mask_list[b]), np.float32))
        else:
            masks.append(np.zeros((1, 128, 128), np.float32))
    return plan, masks


def _build_nc(plan, n_masks):
    from contextlib import ExitStack

    nc = bacc.Bacc(None, target_bir_lowering=False, debug=False)
    MULT = mybir.AluOpType.mult

    xt_d = nc.dram_tensor("xt", [128, ND * T], BF16, kind="ExternalInput")
    wq0_d = nc.dram_tensor("wq0", [128, 2 * D], BF16, kind="ExternalInput")
    wq1_d = nc.dram_tensor("wq1", [128, 2 * D], BF16, kind="ExternalInput")
    wkv_d = nc.dram_tensor("wkv", [128, 2 * D], BF16, kind="ExternalInput")
    wo_d = nc.dram_tensor("wo", [128, G * D], BF16, kind="ExternalInput")
    rtq_d = nc.dram_tensor("rtq", [128, 4 * T], BF16, kind="ExternalInput")
    rtk_d = nc.dram_tensor("rtk", [128, 4 * T], BF16, kind="ExternalInput")
    nm = max(n_masks, 1)
    mpk_d = nc.dram_tensor("mpk", [128, nm * 128], BF16, kind="ExternalInput")
    iden_d = nc.dram_tensor("iden", [128, 128], BF16, kind="ExternalInput")
    cstb_d = nc.dram_tensor("cstb", [128, 4], BF16, kind="ExternalInput")
    cst_d = nc.dram_tensor("cst", [128, 2], F32, kind="ExternalInput")
    bc_d = nc.dram_tensor("bcw", [2, 128], F32R, kind="ExternalInput")
    out_d = nc.dram_tensor("out", [T, D], BF16, kind="ExternalOutput")

    es = ExitStack()
    with es:
        es.enter_context(nc.allow_low_precision("bf16 kernel"))
        tc = es.enter_context(tile.TileContext(nc))
        pool = lambda *a, **k: es.enter_context(tc.tile_pool(*a, **k))
        pp = pool(name="persist", bufs=1)

        # ---------------- persistent SBUF tiles ----------------
        xt = pp.tile([128, ND * T], BF16, tag="xt", name="xt")
        wq0 = pp.tile([128, 2 * D], BF16, tag="wq0", name="wq0")
        wq1 = pp.tile([128, 2 * D], BF16, tag="wq1", name="wq1")
        wkv = pp.tile([128, 2 * D], BF16, tag="wkv", name="wkv")
        wo = pp.tile([128, G * D], BF16, tag="wo", name="wo")
        rtq = pp.tile([128, 4 * T], BF16, tag="rtq", name="rtq")
        rtk = pp.tile([128, 4 * T], BF16, tag="rtk", name="rtk")
        mpk = pp.tile([128, nm * 128], BF16, tag="mpk", name="mpk")
        iden = pp.tile([128, 128], BF16, tag="iden", name="iden")
        cstb = pp.tile([128, 4], BF16, tag="cstb", name="cstb")
        cst = pp.tile([128, 2], F32, tag="cst", name="cst")
        bcw = pp.tile([2, 128], F32R, tag="bcw", name="bcw")
        qh = [pp.tile([128, T], BF16, tag=f"qh{g}", name=f"qh{g}") for g in range(G)]
        kTn = pp.tile([128, T], BF16, tag="kTn", name="kTn")
        V = pp.tile([128, T], BF16, tag="V", name="V")
        vt_sb = pp.tile([128, T], BF16, tag="vt", name="vt")
        sqk = pp.tile([128, T], BF16, tag="sqk", name="sqk")
        qkvh = [pp.tile([128, T], BF16, tag=f"qkvh{g}", name=f"qkvh{g}")
                for g in range(G)]
        rk = pp.tile([1, T], F32, tag="rk", name="rk")
        rkb = pp.tile([128, T], F32, tag="rkb", name="rkb")

        onesb = cstb[:, 2:3]          # bf16 ones column (den / ksum lhsT)

        # ---------------- DMAs (sync queue, priority order) ----------------
        def xchunk(ci):
            sl = slice(ci * 2 * T, (ci + 1) * 2 * T)
            nc.sync.dma_start(xt[:, sl], xt_d[:, sl])

        nc.sync.dma_start(xt[:, 0:T], xt_d[:, 0:T])                # di0
        nc.sync.dma_start(wq0[:, 0:512], wq0_d[:, 0:512])          # f0 di0-3
        nc.sync.dma_start(xt[:, T:2 * T], xt_d[:, T:2 * T])        # di1
        nc.sync.dma_start(wq0[:, D:D + 512], wq0_d[:, D:D + 512])  # f2 di0-3
        xchunk(1)
        nc.sync.dma_start(wq0[:, 512:D], wq0_d[:, 512:D])          # f0 di4-15
        xchunk(2)
        nc.sync.dma_start(wq0[:, D + 512:2 * D], wq0_d[:, D + 512:2 * D])
        xchunk(3)
        for t_, d_ in [(cstb, cstb_d), (cst, cst_d), (bcw, bc_d),
                       (iden, iden_d)]:
            nc.sync.dma_start(t_[:], d_[:])
        for ci in range(4, 8):
            xchunk(ci)
        nc.sync.dma_start(wq1[:], wq1_d[:])
        nc.sync.dma_start(rtq[:], rtq_d[:])
        nc.sync.dma_start(wkv[:], wkv_d[:])
        nc.sync.dma_start(rtk[:], rtk_d[:])
        nc.sync.dma_start(wo[:], wo_d[:])
        nc.sync.dma_start(mpk[:], mpk_d[:])

        # ---------------- stream pools ----------------
        sbra = pool(name="sb_ra", bufs=5)          # rope outputs ra/rb
        sbb = pool(name="sb_bps", bufs=3)          # bps broadcast sbuf
        sbP = pool(name="sb_P", bufs=8)            # attention P tiles
        es1 = ExitStack()
        pool1 = lambda *a, **k: es1.enter_context(tc.tile_pool(*a, **k))
        sbpc = pool1(name="sb_pc", bufs=8)         # psum->sbuf proj copies
        sbm = pool1(name="sb_m", bufs=2)           # rope temporaries
        sbq = pool1(name="sb_sq", bufs=6)          # square tiles
        sbr = pool1(name="sb_rstd", bufs=2)        # rstd / stmp tiles

        psproj = pool1(name="ps_proj", bufs=4, space="PSUM")
        ps_sm = pool1(name="ps_small", bufs=2, space="PSUM")
        ps_bps = pool1(name="ps_bps", bufs=1, space="PSUM")

        # PE p-state warmup: matmuls on an uninitialized scratch tile (values
        # never read) while the input DMAs stream in
        wsc = sbm.tile([128, 128], BF16, tag="m1", name="wscratch")
        nc.gpsimd.memset(wsc[:], 0.0)
        warm = psproj.tile([128, 512], F32, tag="proj", name="warm")
        for _ in range(18):
            nc.tensor.matmul(warm[:, 0:128], wsc[:], wsc[:],
                             start=True, stop=True)
        wdmy = sbr.tile([1, 2], F32, tag="dmy", name="wdmy")
        nc.scalar.copy(wdmy[:], warm[0:1, 0:2])

        def proj_pass(w, feats, inserts=None, fillers=0, pre_pss=None):
            """d-outer pass over `feats` = list of (col_off, psum pair).
            inserts: {di: fn} PE-stream injections. fillers: p-state keepalive
            matmuls per di boundary while the xt stream is still arriving."""
            pss = dict(pre_pss) if pre_pss else {}
            for fo, _ in feats:
                for c in range(2):
                    if (fo, c) not in pss:
                        pss[(fo, c)] = psproj.tile([128, 512], F32, tag="proj",
                                                   name="proj")
            for di in range(ND):
                if inserts and di in inserts:
                    inserts[di]()
                if fillers and di < 8:
                    for _ in range(fillers):
                        nc.tensor.matmul(warm[:, 128:256], wsc[:], wsc[:],
                                         start=True, stop=True)
                for fo, _ in feats:
                    for c in range(2):
                        nc.tensor.matmul(
                            pss[(fo, c)][:],
                            w[:, fo + di * 128: fo + (di + 1) * 128],
                            xt[:, di * T + c * 512: di * T + (c + 1) * 512],
                            start=(di == 0), stop=(di == ND - 1))
            return pss

        def rope_q(pca, pcb, ra, rb, cs):
            m1 = sbm.tile([128, 512], BF16, tag="m1", name="m1")
            m2 = sbm.tile([128, 512], BF16, tag="m2", name="m2")
            qA, qB = rtq[:, 0 * T:1 * T], rtq[:, 1 * T:2 * T]
            qC, qD = rtq[:, 2 * T:3 * T], rtq[:, 3 * T:4 * T]
            nc.vector.tensor_mul(m1[:], pca[:], qA[:, cs])
            nc.vector.tensor_mul(m2[:], pcb[:], qB[:, cs])
            nc.vector.tensor_sub(ra[:], m1[:], m2[:])
            nc.vector.tensor_mul(m1[:], pcb[:], qC[:, cs])
            nc.vector.tensor_mul(m2[:], pca[:], qD[:, cs])
            nc.vector.tensor_add(rb[:], m1[:], m2[:])

        # =========== phase 1: q0 pass ===========
        pq0 = proj_pass(wq0, [(0, None), (D, None)])
        # Act: psum -> sbuf bf16 copies + squares
        pcs0, sqs0 = {}, {}
        for c in range(2):
            for fi, fo in enumerate((0, D)):
                pc = sbpc.tile([128, 512], BF16, tag="pc", name="pc")
                nc.scalar.copy(pc[:], pq0[(fo, c)][:])
                pcs0[(fi, c)] = pc
        for c in range(2):
            for fi in range(2):
                sq = sbq.tile([128, 512], BF16, tag="sq", name="sq")
                nc.scalar.square(sq[:], pcs0[(fi, c)][:])
                sqs0[(fi, c)] = sq
        # DVE: rope p0
        rr0 = {}
        for c in range(2):
            cs = slice(c * 512, (c + 1) * 512)
            ra = sbra.tile([128, 512], BF16, tag="ra", name="ra")
            rb = sbra.tile([128, 512], BF16, tag="rb", name="rb")
            rope_q(pcs0[(0, c)], pcs0[(1, c)], ra, rb, cs)
            rr0[c] = (ra, rb)

        # =========== q1 pass with rstd-p0 insertions ===========
        ss0 = {c: ps_sm.tile([2, 512], F32, tag="ss", name="ss") for c in range(2)}

        def ins_ssq0():
            for c in range(2):
                for fi in range(2):
                    nc.tensor.matmul(ss0[c][:], cstb[:, 0:2], sqs0[(fi, c)][:],
                                     start=(fi == 0), stop=(fi == 1))

        rstd0 = {}
        for c in range(2):
            rstd0[c] = sbr.tile([2, 512], F32R, tag="rstd", name="rstd")

        pq1 = proj_pass(wq1, [(0, None), (D, None)],
                        inserts={10: ins_ssq0},
                        pre_pss={(0, 0): ps_bps.tile([128, 512], F32,
                                                     tag="bps", name="q1pre")})

        # Act: sqrt ss p0 ; DVE: recip -> rstd0
        for c in range(2):
            st = sbr.tile([2, 512], F32, tag="stmp", name="stmp")
            nc.scalar.activation(st[:], ss0[c][:], SQRTF,
                                 bias=cst[0:2, 1:2], scale=float(1.0 / H))
            nc.vector.reciprocal(rstd0[c][:], st[:])
        # PE: bc p0 broadcast matmuls (after rstd0 writers are emitted)
        bps0 = {}
        for c in range(2):
            bp = ps_bps.tile([128, 512], F32, tag="bps", name="bps")
            nc.tensor.matmul(bp[:], bcw[:], rstd0[c][:], start=True, stop=True)
            bps0[c] = bp
        # Act: bps copies ; DVE: qh p0 mults
        for c in range(2):
            bsb = sbb.tile([128, 512], BF16, tag="bsb", name="bsb")
            nc.scalar.copy(bsb[:], bps0[c][:])
            cs = slice(c * 512, (c + 1) * 512)
            ra, rb = rr0[c]
            nc.vector.tensor_mul(qh[0][0:64, cs], ra[0:64, :], bsb[0:64, :])
            nc.vector.tensor_mul(qh[0][64:128, cs], rb[0:64, :], bsb[0:64, :])
            nc.vector.tensor_mul(qh[1][0:64, cs], ra[64:128, :], bsb[64:128, :])
            nc.vector.tensor_mul(qh[1][64:128, cs], rb[64:128, :], bsb[64:128, :])

        # Act: q1 copies + squares ; DVE: rope p1
        pcs1, sqs1 = {}, {}
        for c in range(2):
            for fi, fo in enumerate((0, D)):
                pc = sbpc.tile([128, 512], BF16, tag="pc", name="pc")
                nc.scalar.copy(pc[:], pq1[(fo, c)][:])
                pcs1[(fi, c)] = pc
        for c in range(2):
            for fi in range(2):
                sq = sbq.tile([128, 512], BF16, tag="sq", name="sq")
                nc.scalar.square(sq[:], pcs1[(fi, c)][:])
                sqs1[(fi, c)] = sq
        rr1 = {}
        for c in range(2):
            cs = slice(c * 512, (c + 1) * 512)
            ra = sbra.tile([128, 512], BF16, tag="ra", name="ra")
            rb = sbra.tile([128, 512], BF16, tag="rb", name="rb")
            rope_q(pcs1[(0, c)], pcs1[(1, c)], ra, rb, cs)
            rr1[c] = (ra, rb)

        # =========== k pass with ssq-p1 insertion ===========
        ss1 = {c: ps_sm.tile([2, 512], F32, tag="ss", name="ss") for c in range(2)}

        def ins_ssq1():
            for c in range(2):
                for fi in range(2):
                    nc.tensor.matmul(ss1[c][:], cstb[:, 0:2], sqs1[(fi, c)][:],
                                     start=(fi == 0), stop=(fi == 1))

        pk = proj_pass(wkv, [(0, None)], inserts={12: ins_ssq1})

        rstd1 = {c: sbr.tile([2, 512], F32R, tag="rstd", name="rstd")
                 for c in range(2)}
        for c in range(2):
            st = sbr.tile([2, 512], F32, tag="stmp", name="stmp")
            nc.scalar.activation(st[:], ss1[c][:], SQRTF,
                                 bias=cst[0:2, 1:2], scale=float(1.0 / H))
            nc.vector.reciprocal(rstd1[c][:], st[:])

        # Act: pck copies + sqk squares ; DVE: k rope
        pck = {}
        for c in range(2):
            pc = sbpc.tile([128, 512], BF16, tag="pc", name="pc")
            nc.scalar.copy(pc[:], pk[(0, c)][:])
            pck[c] = pc
            cs = slice(c * 512, (c + 1) * 512)
            nc.scalar.square(sqk[:, cs], pc[:])
        kA, kB = rtk[:, 0 * T:1 * T], rtk[:, 1 * T:2 * T]
        kC, kD = rtk[:, 2 * T:3 * T], rtk[:, 3 * T:4 * T]

        def k_rope(c):
            cs = slice(c * 512, (c + 1) * 512)
            m1 = sbm.tile([64, 512], BF16, tag="km1", name="km1")
            m2 = sbm.tile([64, 512], BF16, tag="km2", name="km2")
            k0, k1 = pck[c][0:64, :], pck[c][64:128, :]
            nc.vector.tensor_mul(m1[:], k0, kA[0:64, cs])
            nc.vector.tensor_mul(m2[:], k1, kB[64:128, cs])
            nc.vector.tensor_sub(kTn[0:64, cs], m1[:], m2[:])
            nc.vector.tensor_mul(m1[:], k1, kC[64:128, cs])
            nc.vector.tensor_mul(m2[:], k0, kD[0:64, cs])
            nc.vector.tensor_add(kTn[64:128, cs], m1[:], m2[:])

        k_rope(0)

        # =========== v pass with ksum / bc-p1 insertions ===========
        ksum = {c: ps_sm.tile([2, 512], F32, tag="ss", name="ks") for c in range(2)}
        bps1 = {}

        def ins_ksum(c):
            def f():
                nc.tensor.matmul(ksum[c][0:1, :], onesb,
                                 sqk[:, c * 512:(c + 1) * 512],
                                 start=True, stop=True)
            return f

        def ins_bc1():
            for c in range(2):
                bp = ps_bps.tile([128, 512], F32, tag="bps", name="bps")
                nc.tensor.matmul(bp[:], bcw[:], rstd1[c][:], start=True, stop=True)
                bps1[c] = bp

        units = []           # (g, tt) in half-major, g-major, tt-minor order
        for half in range(2):
            tts = [tt for tt in range(half * 4, (half + 1) * 4) if plan[tt]]
            for g in range(G):
                for tt in tts:
                    units.append((g, tt, half))
        udata = {}

        def emit_lg_exp_mask(u, lg_pool=None):
            g, tt, half = u
            ents = plan[tt]
            n_e = len(ents)
            w = n_e * 128
            lg = (lg_pool or ps_lg).tile([128, 512], F32, tag="proj" if lg_pool else "lg", name="lg")
            for ei, (si, kind, mi) in enumerate(ents):
                nc.tensor.matmul(lg[:, ei * 128:(ei + 1) * 128],
                                 kTn[:, si * 128:(si + 1) * 128],
                                 qh[g][:, tt * 128:(tt + 1) * 128],
                                 start=True, stop=True)
            P = sbP.tile([128, 512], BF16, tag="P", name="P")
            nc.scalar.activation(P[:, :w], lg[:, :w], EXPF, scale=1.0)
            spans = []
            for ei, (si, kind, mi) in enumerate(ents):
                if kind != "partial":
                    continue
                if spans and spans[-1][1] == ei:
                    spans[-1][1] = ei + 1
                else:
                    spans.append([ei, ei + 1, mi])
            for e0, e1, mi0 in spans:
                nc.vector.tensor_mul(
                    P[:, e0 * 128:e1 * 128], P[:, e0 * 128:e1 * 128],
                    mpk[:, mi0 * 128:(mi0 + (e1 - e0)) * 128])
            udata[u] = (P, n_e)

        _pre_units = [u for u in units if u[2] == 0][:4]
        _pre_done = list(_pre_units)

        # v chunk 0 first: its psums stop early so V s-tiles 0..3 are
        # transposed + copied while v chunk 1 still projects
        pv0 = {}
        for di in range(ND):
            if di == 6:
                ins_ksum(0)()
            if di == 10:
                ins_bc1()
            if di == 9:
                ins_ksum(1)()
            if di == 0:
                pv0[0] = psproj.tile([128, 512], F32, tag="proj", name="proj")
            nc.tensor.matmul(pv0[0][:], wkv[:, D + di * 128:D + (di + 1) * 128],
                             xt[:, di * T: di * T + 512],
                             start=(di == 0), stop=(di == ND - 1))
        pv = {(D, 0): pv0[0]}

        # Act: sqrt ksum (SCALE*rstd_k = 1/sqrt(ssq + H*eps))
        kst = {}
        for c in range(2):
            st = sbr.tile([1, 512], F32, tag="kst", name="kst")
            nc.scalar.activation(st[:], ksum[c][0:1, :], SQRTF,
                                 bias=cst[0:1, 0:1], scale=1.0)
            kst[c] = st
        # DVE: per-chunk recip -> Pool broadcast -> fold; chunk-0 chain runs
        # before k_rope(1) so the first attention t-tiles unblock early
        def k_fold(c):
            cs = slice(c * 512, (c + 1) * 512)
            nc.vector.reciprocal(rk[0:1, cs], kst[c][:])
            nc.gpsimd.partition_broadcast(rkb[:, cs], rk[0:1, cs])
            nc.vector.tensor_mul(kTn[:, cs], kTn[:, cs], rkb[:, cs])

        k_fold(0)
        k_rope(1)
        k_fold(1)

        # dummy exp: forces the exp act-table load into this idle window,
        # before the first real attention exp
        dmy = sbr.tile([1, 2], F32, tag="dmy", name="dmy")
        nc.scalar.activation(dmy[:], kst[1][0:1, 0:2], EXPF, scale=1.0)

        # Act: bps p1 copies ; DVE (deferred into att loop): qh p1 mults
        bsb1 = {}
        for c in range(2):
            bsb = sbb.tile([128, 512], BF16, tag="bsb", name="bsb")
            nc.scalar.copy(bsb[:], bps1[c][:])
            bsb1[c] = bsb

        def qh_p1_mults(c):
            cs = slice(c * 512, (c + 1) * 512)
            ra, rb = rr1[c]
            bsb = bsb1[c]
            nc.vector.tensor_mul(qh[2][0:64, cs], ra[0:64, :], bsb[0:64, :])
            nc.vector.tensor_mul(qh[2][64:128, cs], rb[0:64, :], bsb[0:64, :])
            nc.vector.tensor_mul(qh[3][0:64, cs], ra[64:128, :], bsb[64:128, :])
            nc.vector.tensor_mul(qh[3][64:128, cs], rb[64:128, :], bsb[64:128, :])

        # Act: vt c0 copy ; PE: transpose + V copies for s-tiles 0..3
        nc.scalar.copy(vt_sb[:, 0:512], pv[(D, 0)][:])

        def vt_group(j0, j1):
            for j in range(j0, j1):
                vp = ps_bps.tile([128, 128], BF16, tag="vtpe", name="vtpe")
                nc.tensor.transpose(vp[:], vt_sb[:, j * 128:(j + 1) * 128], iden[:])
                nc.scalar.copy(V[:, j * 128:(j + 1) * 128], vp[:])

        # v chunk 1 pass (Vt c0 transposes + early att logit units overlap)
        pv1 = psproj.tile([128, 512], F32, tag="proj", name="proj")
        for di in range(ND):
            if di == 4:
                vt_group(0, 2)
            if di == 8:
                vt_group(2, 4)
            if di >= 8 and _pre_units:
                emit_lg_exp_mask(_pre_units.pop(0), lg_pool=psproj)
            nc.tensor.matmul(pv1[:], wkv[:, D + di * 128:D + (di + 1) * 128],
                             xt[:, di * T + 512: di * T + 1024],
                             start=(di == 0), stop=(di == ND - 1))
        nc.scalar.copy(vt_sb[:, 512:1024], pv1[:])

        # =========== phase 2: attention + out projection ===========
        es1.close()   # free phase-1 SBUF + PSUM
        sbrec = pool(name="sb_rec", bufs=4)        # quad reciprocals
        sbbc = pool(name="sb_bcs", bufs=4)         # quad broadcasts
        obp = pool(name="sb_ob", bufs=3)           # output staging
        ps_lg = pool(name="ps_lg", bufs=2, space="PSUM")
        ps_qkv = pool(name="ps_qkv", bufs=2, space="PSUM")
        ps_den = pool(name="ps_den", bufs=1, space="PSUM")
        ps_op = pool(name="ps_op", bufs=3, space="PSUM")

        def vt_group2(j0, j1):
            for j in range(j0, j1):
                vp = ps_op.tile([128, 128], BF16, tag="op", name="vtp")
                nc.tensor.transpose(vp[:], vt_sb[:, j * 128:(j + 1) * 128], iden[:])
                nc.scalar.copy(V[:, j * 128:(j + 1) * 128], vp[:])


        quad_ps = {}         # (g, half) -> (den_ps, qkv_ps, n_done, n_total)
        quad_cnt = {}
        for g, tt, half in units:
            quad_cnt[(g, half)] = quad_cnt.get((g, half), 0) + 1

        udata = {}


        def emit_den_qkv(u):
            g, tt, half = u
            qk = quad_ps.get((g, half))
            if qk is None:
                den = ps_den.tile([1, 512], F32, tag="den", name="den")
                qkv = ps_qkv.tile([128, 512], F32, tag="qkv", name="qkv")
                qk = quad_ps[(g, half)] = [den, qkv, 0]
            den, qkv, _ = qk
            P, n_e = udata.pop(u)
            ents = plan[tt]
            ttl = tt - half * 4
            for ei, (si, kind, mi) in enumerate(ents):
                nc.tensor.matmul(den[0:1, ttl * 128:(ttl + 1) * 128], onesb,
                                 P[:, ei * 128:(ei + 1) * 128],
                                 start=(ei == 0), stop=(ei == n_e - 1))
            for ei, (si, kind, mi) in enumerate(ents):
                nc.tensor.matmul(qkv[:, ttl * 128:(ttl + 1) * 128],
                                 V[:, si * 128:(si + 1) * 128],
                                 P[:, ei * 128:(ei + 1) * 128],
                                 start=(ei == 0), stop=(ei == n_e - 1))
            qk[2] += 1
            if qk[2] == quad_cnt[(g, half)]:
                hs = slice(half * 512, (half + 1) * 512)
                rec = sbrec.tile([1, 512], F32, tag="rec", name="rec")
                nc.vector.reciprocal(rec[:], den[0:1, :])
                bcs = sbbc.tile([128, 512], F32, tag="bcs", name="bcs")
                nc.gpsimd.partition_broadcast(bcs[:], rec[0:1, :])
                h0_ = half * 512
                nc.vector.tensor_mul(qkvh[g][:, h0_:h0_ + 256],
                                     qkv[:, 0:256], bcs[:, 0:256])
                nc.vector.tensor_mul(qkvh[g][:, h0_ + 256:h0_ + 512],
                                     qkv[:, 256:512], bcs[:, 256:512])
                del quad_ps[(g, half)]

        obs = {}

        def emit_outproj(tt, dc, w512=512):
            op = ps_op.tile([128, 512], F32, tag="op", name="op")
            for g in range(G):
                nc.tensor.matmul(op[:, 0:w512],
                                 qkvh[g][:, tt * 128:(tt + 1) * 128],
                                 wo[:, g * D + dc * 512: g * D + dc * 512 + w512],
                                 start=(g == 0), stop=(g == G - 1))
            ob = obs.get(tt)
            if ob is None:
                ob = obs[tt] = obp.tile([128, D], BF16, tag="ob", name="ob")
            if dc % 2 == 0:
                nc.vector.tensor_copy(ob[:, dc * 512:(dc + 1) * 512], op[:])
            else:
                nc.scalar.copy(ob[:, dc * 512:(dc + 1) * 512], op[:])
            if tt >= 4:
                q = (nc.sync, nc.scalar, nc.gpsimd, nc.sync)[dc]
                q.dma_start(
                    out_d[tt * 128:(tt + 1) * 128, dc * 512:(dc + 1) * 512],
                    ob[:, dc * 512:(dc + 1) * 512])
                if dc == 3:
                    del obs[tt]
            elif dc == 3:
                nc.sync.dma_start(out_d[tt * 128:(tt + 1) * 128, :], ob[:])
                del obs[tt]

        # --- attention half 0, software-pipelined depth 2 ---
        h0_units = [u for u in units if u[2] == 0]
        h1_units = [u for u in units if u[2] == 1]

        LAG = 3
        seq = []
        for i, u in enumerate(h0_units):
            if u not in _pre_done:
                seq.append(("lg", u))
            if i == 1:
                seq.append(("vt", (4, 6)))
            if i == 2:
                seq.append(("vt", (6, 8)))
                seq.append(("qhp1", 0))
            if i == len(h0_units) - 1:
                seq.append(("qhp1", 1))
            if i >= LAG:
                seq.append(("dq", h0_units[i - LAG]))
        for u in h0_units[-LAG:]:
            seq.append(("dq", u))
        # half 1 attention interleaved with half-0 out projection
        op_h0 = [(tt, dc) for tt in range(0, 4) for dc in range(4)]
        op_h1 = [(tt, dc) for tt in range(4, 8) for dc in range(4)]
        opi = 0
        for i, u in enumerate(h1_units):
            seq.append(("lg", u))
            if i >= LAG:
                seq.append(("dq", h1_units[i - LAG]))
            if i >= 4 and opi < len(op_h0):
                seq.append(("op", op_h0[opi]))
                opi += 1
        for u in h1_units[-LAG:]:
            seq.append(("dq", u))
        for rest in op_h0[opi:]:
            seq.append(("op", rest))
        for o in op_h1:
            seq.append(("op", o))

        for kind, arg in seq:
            if kind == "lg":
                emit_lg_exp_mask(arg)
            elif kind == "dq":
                emit_den_qkv(arg)
            elif kind == "vt":
                vt_group2(*arg)
            elif kind == "qhp1":
                qh_p1_mults(arg)
            elif kind == "op":
                emit_outproj(*arg)

    nc.finalize()
    return nc


_CACHE = {}


def kernel(x, segment_ids, Wq, Wk, Wv, Wo, q_scale, k_scale):
    global LAST_RESULTS
    import os
    import ml_dtypes

    bf = ml_dtypes.bfloat16
    x = np.asarray(x, np.float32)
    seg = np.asarray(segment_ids)
    Wq = np.asarray(Wq, np.float32)
    Wk = np.asarray(Wk, np.float32)
    Wv = np.asarray(Wv, np.float32)
    Wo = np.asarray(Wo, np.float32)
    q_scale = np.asarray(q_scale, np.float64)
    k_scale = np.asarray(k_scale, np.float64)

    plan, masks = _classify([seg[b] for b in range(B)])
    key = repr(plan)
    if key not in _CACHE:
        _CACHE[key] = _build_nc(plan, masks[0].shape[0])
    nc = _CACHE[key]

    half = H // 2
    timescale = ROPE_BASE ** (2.0 * np.arange(half, dtype=np.float64) / H)
    qs_lo = np.tile(q_scale[:64], 2)[:, None]
    qs_hi = np.tile(q_scale[64:], 2)[:, None]
    ks_lo = np.tile(k_scale[:64], 2)[:, None]
    ks_hi = np.tile(k_scale[64:], 2)[:, None]
    rtq_b, rtk_b = [], []
    for b in range(B):
        pos = _positions(seg[b])
        sinus = pos[None, :] / timescale[:, None]        # [64, T]
        sd = np.vstack([np.sin(sinus)] * 2)              # [128, T]
        cd = np.vstack([np.cos(sinus)] * 2)
        rtq_b.append(np.hstack([qs_lo * cd, qs_hi * sd, qs_hi * cd, qs_lo * sd]
                               ).astype(bf))
        rtk_b.append(np.hstack([ks_lo * cd, ks_hi * sd, ks_hi * cd, ks_lo * sd]
                               ).astype(bf))

    cstb = np.zeros((128, 4), np.float32)
    cstb[0:64, 0] = 1.0
    cstb[64:128, 1] = 1.0
    cstb[:, 2] = 1.0
    cst = np.zeros((128, 2), np.float32)
    cst[:, 0] = H * EPS
    cst[:, 1] = EPS
    bcw = np.zeros((2, 128), np.float32)
    bcw[0, 0:64] = 1.0
    bcw[1, 64:128] = 1.0
    iden = np.eye(128, dtype=np.float32)

    in_maps = []
    for core in range(8):
        b, kv = core // K, core % K
        xt = np.ascontiguousarray(
            x[b].T.reshape(ND, 128, T).transpose(1, 0, 2).reshape(128, ND * T))

        def qfeat(w, cols):
            # [D, 128] -> [128(d_lo), ND*128] with w[p, di*128+j] = W[di*128+p, cols[j]]
            sub = w[:, cols]                             # [D, 128]
            return sub.reshape(ND, 128, 128).transpose(1, 0, 2).reshape(128, D)

        base = kv * 4 * H
        f_cols = []
        for pair in range(2):      # (f0,f2) then (f1,f3)
            for hv in range(2):
                cols = np.concatenate([
                    np.arange(base + (2 * g4 + pair) * H + hv * 64,
                              base + (2 * g4 + pair) * H + hv * 64 + 64)
                    for g4 in range(2)])
                f_cols.append(cols)
        # heads order per pair: pair0 -> heads (0,1)?  cols above pick heads
        # (pair + 2*g4): pair0 -> heads 0,2 ... fix: want pair0 = heads 0,1.
        f_cols = []
        for pair, heads in [(0, (0, 1)), (1, (2, 3))]:
            for hv in range(2):
                cols = np.concatenate([
                    np.arange(base + g4 * H + hv * 64,
                              base + g4 * H + hv * 64 + 64) for g4 in heads])
                f_cols.append(cols)
        wq0 = np.hstack([qfeat(Wq, f_cols[0]), qfeat(Wq, f_cols[1])]).astype(bf)
        wq1 = np.hstack([qfeat(Wq, f_cols[2]), qfeat(Wq, f_cols[3])]).astype(bf)
        kcols = np.arange(kv * H, (kv + 1) * H)
        wkv = np.hstack([qfeat(Wk, kcols), qfeat(Wv, kcols)]).astype(bf)
        wo_t = np.ascontiguousarray(
            Wo[kv * 512:(kv + 1) * 512].reshape(G, 128, D)
            .transpose(1, 0, 2).reshape(128, G * D)).astype(bf)
        nm = max(masks[b].shape[0], 1)
        mpk = np.ascontiguousarray(
            masks[b].transpose(1, 0, 2).reshape(128, nm * 128)).astype(bf)

        in_maps.append({
            "xt": xt.astype(bf), "wq0": wq0, "wq1": wq1, "wkv": wkv,
            "wo": wo_t, "rtq": rtq_b[b], "rtk": rtk_b[b], "mpk": mpk,
            "iden": iden.astype(bf), "cstb": cstb.astype(bf),
            "cst": cst, "bcw": bcw,
        })

    do_trace = os.environ.get("BASS_TRACE") == "1"
    res = run_bass_kernel_spmd(
        nc, in_maps, core_ids=list(range(8)), trace=do_trace)
    LAST_RESULTS = res

    out = np.zeros((B, T, D), np.float32)
    for core in range(8):
        out[core // K] += res.results[core]["out"].astype(np.float32)
    return out


# revision 5
# speedup vs baseline: 1.0291x; 1.0173x over previous
"""Trainium2 Bass kernel v2 for segment-causal GQA attention.

Sharding: 8 cores = batch (2) x kv-head (4), as baseline. All matmul
operands bf16 (1 cyc/row at any free size), halving DMA and enabling
128-wide attention t-tiles. RMS scales folded into host rope tables;
SCALE*rstd_k folded into kTn columns so exp batches over s-tiles.
Attention: per (g, t-tile) unit = n_e logit matmuls into one psum run,
one exp, one mask mult, n_e den + n_e qkv matmuls; den/qkv batched
per (g, 4-tt quad) for one reciprocal + broadcast + normalize.
"""

import sys

sys.path.insert(0, "/opt/trn_rl_repo")

import numpy as np

import concourse.bacc as bacc
import concourse.bass as bass  # noqa: F401
import concourse.tile as tile
from concourse import mybir
from concourse.bass_utils import run_bass_kernel_spmd

B, T, D = 2, 1024, 2048
N, K, H = 16, 4, 128
G = N // K
EPS = 1e-6
ROPE_BASE = 10000.0
ND = D // 128        # 16 d-tiles
NS = T // 128        # 8 s-tiles
NT = T // 128        # 8 t-tiles
F32 = mybir.dt.float32
F32R = mybir.dt.float32r
BF16 = mybir.dt.bfloat16
EXPF = mybir.ActivationFunctionType.Exp
SQRTF = mybir.ActivationFunctionType.Sqrt

LAST_RESULTS = None  # test harness reads exec_time_ns from here


def _positions(seg):
    t = seg.shape[0]
    idx = np.arange(t, dtype=np.int64)
    is_start = np.concatenate([[True], seg[1:] != seg[:-1]])
    seg_start = np.maximum.accumulate(np.where(is_start, idx, 0))
    return (idx - seg_start).astype(np.float64)


def _classify(seg_rows):
    """Union tile classification over batches at 128x128 granularity.

    Returns (plan, masks_per_batch): plan[tt] = list of (si, kind, mask_idx);
    masks_per_batch[b] = float32 [max(n_masks,1), 128, 128] of 0/1.
    """
    idx = np.arange(T)
    valids = []
    for b in range(len(seg_rows)):
        seg = seg_rows[b]
        valids.append((seg[:, None] == seg[None, :]) & (idx[:, None] <= idx[None, :]))
    plan = []
    mask_list = [[] for _ in range(len(seg_rows))]
    n_masks = 0
    for tt in range(NT):
        t0 = tt * 128
        entries = []
        for si in range(NS):
            s0 = si * 128
            subs = [v[s0:s0 + 128, t0:t0 + 128] for v in valids]
            if not any(s.any() for s in subs):
                continue
            if all(s.all() for s in subs):
                entries.append((si, "full", -1))
            else:
                for b in range(len(seg_rows)):
                    mask_list[b].append(subs[b].astype(np.float32))
                entries.append((si, "partial", n_masks))
                n_masks += 1
        plan.append(entries)
    masks = []
    for b in range(len(seg_rows)):
        if n_masks:
            masks.append(np.ascontiguousarray(np.stack(mask_list[b]), np.float32))
        else:
            masks.append(np.zeros((1, 128, 128), np.float32))
    return plan, masks


def _build_nc(plan, n_masks):
    from contextlib import ExitStack

    nc = bacc.Bacc(None, target_bir_lowering=False, debug=False)
    MULT = mybir.AluOpType.mult

    xt_d = nc.dram_tensor("xt", [128, ND * T], BF16, kind="ExternalInput")
    wq0_d = nc.dram_tensor("wq0", [128, 2 * D], BF16, kind="ExternalInput")
    wq1_d = nc.dram_tensor("wq1", [128, 2 * D], BF16, kind="ExternalInput")
    wkv_d = nc.dram_tensor("wkv", [128, 2 * D], BF16, kind="ExternalInput")
    wo_d = nc.dram_tensor("wo", [128, G * D], BF16, kind="ExternalInput")
    rtq_d = nc.dram_tensor("rtq", [128, 4 * T], BF16, kind="ExternalInput")
    rtk_d = nc.dram_tensor("rtk", [128, 4 * T], BF16, kind="ExternalInput")
    nm = max(n_masks, 1)
    mpk_d = nc.dram_tensor("mpk", [128, nm * 128], BF16, kind="ExternalInput")
    iden_d = nc.dram_tensor("iden", [128, 128], BF16, kind="ExternalInput")
    cstb_d = nc.dram_tensor("cstb", [128, 4], BF16, kind="ExternalInput")
    cst_d = nc.dram_tensor("cst", [128, 2], F32, kind="ExternalInput")
    bc_d = nc.dram_tensor("bcw", [2, 128], F32R, kind="ExternalInput")
    out_d = nc.dram_tensor("out", [T, D], BF16, kind="ExternalOutput")

    es = ExitStack()
    with es:
        es.enter_context(nc.allow_low_precision("bf16 kernel"))
        tc = es.enter_context(tile.TileContext(nc))
        pool = lambda *a, **k: es.enter_context(tc.tile_pool(*a, **k))
        pp = pool(name="persist", bufs=1)

        # ---------------- persistent SBUF tiles ----------------
        xt = pp.tile([128, ND * T], BF16, tag="xt", name="xt")
        wq0 = pp.tile([128, 2 * D], BF16, tag="wq0", name="wq0")
        wq1 = pp.tile([128, 2 * D], BF16, tag="wq1", name="wq1")
        wkv = pp.tile([128, 2 * D], BF16, tag="wkv", name="wkv")
        wo = pp.tile([128, G * D], BF16, tag="wo", name="wo")
        rtq = pp.tile([128, 4 * T], BF16, tag="rtq", name="rtq")
        rtk = pp.tile([128, 4 * T], BF16, tag="rtk", name="rtk")
        mpk = pp.tile([128, nm * 128], BF16, tag="mpk", name="mpk")
        iden = pp.tile([128, 128], BF16, tag="iden", name="iden")
        cstb = pp.tile([128, 4], BF16, tag="cstb", name="cstb")
        cst = pp.tile([128, 2], F32, tag="cst", name="cst")
        bcw = pp.tile([2, 128], F32R, tag="bcw", name="bcw")
        qh = [pp.tile([128, T], BF16, tag=f"qh{g}", name=f"qh{g}") for g in range(G)]
        kTn = pp.tile([128, T], BF16, tag="kTn", name="kTn")
        V = pp.tile([128, T], BF16, tag="V", name="V")
        vt_sb = pp.tile([128, T], BF16, tag="vt", name="vt")
        sqk = pp.tile([128, T], BF16, tag="sqk", name="sqk")
        qkvh = [pp.tile([128, T], BF16, tag=f"qkvh{g}", name=f"qkvh{g}")
                for g in range(G)]
        rk = pp.tile([1, T], F32, tag="rk", name="rk")
        rkb = pp.tile([128, T], F32, tag="rkb", name="rkb")

        onesb = cstb[:, 2:3]          # bf16 ones column (den / ksum lhsT)

        # ---------------- DMAs (sync queue, priority order) ----------------
        def xchunk(ci):
            sl = slice(ci * 2 * T, (ci + 1) * 2 * T)
            nc.sync.dma_start(xt[:, sl], xt_d[:, sl])

        nc.sync.dma_start(xt[:, 0:T], xt_d[:, 0:T])                # di0
        nc.sync.dma_start(wq0[:, 0:512], wq0_d[:, 0:512])          # f0 di0-3
        nc.sync.dma_start(xt[:, T:2 * T], xt_d[:, T:2 * T])        # di1
        nc.sync.dma_start(wq0[:, D:D + 512], wq0_d[:, D:D + 512])  # f2 di0-3
        xchunk(1)
        nc.sync.dma_start(wq0[:, 512:D], wq0_d[:, 512:D])          # f0 di4-15
        xchunk(2)
        nc.sync.dma_start(wq0[:, D + 512:2 * D], wq0_d[:, D + 512:2 * D])
        xchunk(3)
        for t_, d_ in [(cstb, cstb_d), (cst, cst_d), (bcw, bc_d),
                       (iden, iden_d)]:
            nc.sync.dma_start(t_[:], d_[:])
        for ci in range(4, 8):
            xchunk(ci)
        nc.sync.dma_start(wq1[:], wq1_d[:])
        nc.sync.dma_start(rtq[:], rtq_d[:])
        nc.sync.dma_start(wkv[:], wkv_d[:])
        nc.sync.dma_start(rtk[:], rtk_d[:])
        nc.sync.dma_start(wo[:], wo_d[:])
        nc.sync.dma_start(mpk[:], mpk_d[:])

        # ---------------- stream pools ----------------
        sbra = pool(name="sb_ra", bufs=5)          # rope outputs ra/rb
        sbb = pool(name="sb_bps", bufs=3)          # bps broadcast sbuf
        sbP = pool(name="sb_P", bufs=8)            # attention P tiles
        es1 = ExitStack()
        pool1 = lambda *a, **k: es1.enter_context(tc.tile_pool(*a, **k))
        sbpc = pool1(name="sb_pc", bufs=8)         # psum->sbuf proj copies
        sbm = pool1(name="sb_m", bufs=2)           # rope temporaries
        sbq = pool1(name="sb_sq", bufs=6)          # square tiles
        sbr = pool1(name="sb_rstd", bufs=2)        # rstd / stmp tiles

        psproj = pool1(name="ps_proj", bufs=4, space="PSUM")
        ps_sm = pool1(name="ps_small", bufs=2, space="PSUM")
        ps_bps = pool1(name="ps_bps", bufs=1, space="PSUM")

        # PE p-state warmup: matmuls on an uninitialized scratch tile (values
        # never read) while the input DMAs stream in
        wsc = sbm.tile([128, 128], BF16, tag="m1", name="wscratch")
        nc.gpsimd.memset(wsc[:], 0.0)
        warm = psproj.tile([128, 512], F32, tag="proj", name="warm")
        for _ in range(18):
            nc.tensor.matmul(warm[:, 0:128], wsc[:], wsc[:],
                             start=True, stop=True)
        wdmy = sbr.tile([1, 2], F32, tag="dmy", name="wdmy")
        nc.scalar.copy(wdmy[:], warm[0:1, 0:2])

        def proj_pass(w, feats, inserts=None, fillers=0, pre_pss=None):
            """d-outer pass over `feats` = list of (col_off, psum pair).
            inserts: {di: fn} PE-stream injections. fillers: p-state keepalive
            matmuls per di boundary while the xt stream is still arriving."""
            pss = dict(pre_pss) if pre_pss else {}
            for fo, _ in feats:
                for c in range(2):
                    if (fo, c) not in pss:
                        pss[(fo, c)] = psproj.tile([128, 512], F32, tag="proj",
                                                   name="proj")
            for di in range(ND):
                if inserts and di in inserts:
                    inserts[di]()
                if fillers and di < 8:
                    for _ in range(fillers):
                        nc.tensor.matmul(warm[:, 128:256], wsc[:], wsc[:],
                                         start=True, stop=True)
                for fo, _ in feats:
                    for c in range(2):
                        nc.tensor.matmul(
                            pss[(fo, c)][:],
                            w[:, fo + di * 128: fo + (di + 1) * 128],
                            xt[:, di * T + c * 512: di * T + (c + 1) * 512],
                            start=(di == 0), stop=(di == ND - 1))
            return pss

        def rope_q(pca, pcb, ra, rb, cs):
            m1 = sbm.tile([128, 512], BF16, tag="m1", name="m1")
            m2 = sbm.tile([128, 512], BF16, tag="m2", name="m2")
            qA, qB = rtq[:, 0 * T:1 * T], rtq[:, 1 * T:2 * T]
            qC, qD = rtq[:, 2 * T:3 * T], rtq[:, 3 * T:4 * T]
            nc.vector.tensor_mul(m1[:], pca[:], qA[:, cs])
            nc.vector.tensor_mul(m2[:], pcb[:], qB[:, cs])
            nc.vector.tensor_sub(ra[:], m1[:], m2[:])
            nc.vector.tensor_mul(m1[:], pcb[:], qC[:, cs])
            nc.vector.tensor_mul(m2[:], pca[:], qD[:, cs])
            nc.vector.tensor_add(rb[:], m1[:], m2[:])

        # =========== phase 1: q0 pass ===========
        pq0 = proj_pass(wq0, [(0, None), (D, None)])
        # Act: psum -> sbuf bf16 copies + squares
        pcs0, sqs0 = {}, {}
        for c in range(2):
            for fi, fo in enumerate((0, D)):
                pc = sbpc.tile([128, 512], BF16, tag="pc", name="pc")
                nc.scalar.copy(pc[:], pq0[(fo, c)][:])
                pcs0[(fi, c)] = pc
        for c in range(2):
            for fi in range(2):
                sq = sbq.tile([128, 512], BF16, tag="sq", name="sq")
                nc.scalar.square(sq[:], pcs0[(fi, c)][:])
                sqs0[(fi, c)] = sq
        # DVE: rope p0
        rr0 = {}
        for c in range(2):
            cs = slice(c * 512, (c + 1) * 512)
            ra = sbra.tile([128, 512], BF16, tag="ra", name="ra")
            rb = sbra.tile([128, 512], BF16, tag="rb", name="rb")
            rope_q(pcs0[(0, c)], pcs0[(1, c)], ra, rb, cs)
            rr0[c] = (ra, rb)

        # =========== q1 pass with rstd-p0 insertions ===========
        ss0 = {c: ps_sm.tile([2, 512], F32, tag="ss", name="ss") for c in range(2)}

        def ins_ssq0():
            for c in range(2):
                for fi in range(2):
                    nc.tensor.matmul(ss0[c][:], cstb[:, 0:2], sqs0[(fi, c)][:],
                                     start=(fi == 0), stop=(fi == 1))

        rstd0 = {}
        for c in range(2):
            rstd0[c] = sbr.tile([2, 512], F32R, tag="rstd", name="rstd")

        pq1 = proj_pass(wq1, [(0, None), (D, None)],
                        inserts={10: ins_ssq0},
                        pre_pss={(0, 0): ps_bps.tile([128, 512], F32,
                                                     tag="bps", name="q1pre")})

        # Act: sqrt ss p0 ; DVE: recip -> rstd0
        for c in range(2):
            st = sbr.tile([2, 512], F32, tag="stmp", name="stmp")
            nc.scalar.activation(st[:], ss0[c][:], SQRTF,
                                 bias=cst[0:2, 1:2], scale=float(1.0 / H))
            nc.vector.reciprocal(rstd0[c][:], st[:])
        # PE: bc p0 broadcast matmuls (after rstd0 writers are emitted)
        bps0 = {}
        for c in range(2):
            bp = ps_bps.tile([128, 512], F32, tag="bps", name="bps")
            nc.tensor.matmul(bp[:], bcw[:], rstd0[c][:], start=True, stop=True)
            bps0[c] = bp
        # Act: bps copies ; DVE: qh p0 mults
        for c in range(2):
            bsb = sbb.tile([128, 512], BF16, tag="bsb", name="bsb")
            nc.scalar.copy(bsb[:], bps0[c][:])
            cs = slice(c * 512, (c + 1) * 512)
            ra, rb = rr0[c]
            nc.vector.tensor_mul(qh[0][0:64, cs], ra[0:64, :], bsb[0:64, :])
            nc.vector.tensor_mul(qh[0][64:128, cs], rb[0:64, :], bsb[0:64, :])
            nc.vector.tensor_mul(qh[1][0:64, cs], ra[64:128, :], bsb[64:128, :])
            nc.vector.tensor_mul(qh[1][64:128, cs], rb[64:128, :], bsb[64:128, :])

        # Act: q1 copies + squares ; DVE: rope p1
        pcs1, sqs1 = {}, {}
        for c in range(2):
            for fi, fo in enumerate((0, D)):
                pc = sbpc.tile([128, 512], BF16, tag="pc", name="pc")
                nc.scalar.copy(pc[:], pq1[(fo, c)][:])
                pcs1[(fi, c)] = pc
        for c in range(2):
            for fi in range(2):
                sq = sbq.tile([128, 512], BF16, tag="sq", name="sq")
                nc.scalar.square(sq[:], pcs1[(fi, c)][:])
                sqs1[(fi, c)] = sq
        rr1 = {}
        for c in range(2):
            cs = slice(c * 512, (c + 1) * 512)
            ra = sbra.tile([128, 512], BF16, tag="ra", name="ra")
            rb = sbra.tile([128, 512], BF16, tag="rb", name="rb")
            rope_q(pcs1[(0, c)], pcs1[(1, c)], ra, rb, cs)
            rr1[c] = (ra, rb)

        # =========== k pass with ssq-p1 insertion ===========
        ss1 = {c: ps_sm.tile([2, 512], F32, tag="ss", name="ss") for c in range(2)}

        def ins_ssq1():
            for c in range(2):
                for fi in range(2):
                    nc.tensor.matmul(ss1[c][:], cstb[:, 0:2], sqs1[(fi, c)][:],
                                     start=(fi == 0), stop=(fi == 1))

        pk = proj_pass(wkv, [(0, None)], inserts={12: ins_ssq1})

        rstd1 = {c: sbr.tile([2, 512], F32R, tag="rstd", name="rstd")
                 for c in range(2)}
        for c in range(2):
            st = sbr.tile([2, 512], F32, tag="stmp", name="stmp")
            nc.scalar.activation(st[:], ss1[c][:], SQRTF,
                                 bias=cst[0:2, 1:2], scale=float(1.0 / H))
            nc.vector.reciprocal(rstd1[c][:], st[:])

        # Act: pck copies + sqk squares ; DVE: k rope
        pck = {}
        for c in range(2):
            pc = sbpc.tile([128, 512], BF16, tag="pc", name="pc")
            nc.scalar.copy(pc[:], pk[(0, c)][:])
            pck[c] = pc
            cs = slice(c * 512, (c + 1) * 512)
            nc.scalar.square(sqk[:, cs], pc[:])
        kA, kB = rtk[:, 0 * T:1 * T], rtk[:, 1 * T:2 * T]
        kC, kD = rtk[:, 2 * T:3 * T], rtk[:, 3 * T:4 * T]

        def k_rope(c):
            cs = slice(c * 512, (c + 1) * 512)
            m1 = sbm.tile([64, 512], BF16, tag="km1", name="km1")
            m2 = sbm.tile([64, 512], BF16, tag="km2", name="km2")
            k0, k1 = pck[c][0:64, :], pck[c][64:128, :]
            nc.vector.tensor_mul(m1[:], k0, kA[0:64, cs])
            nc.vector.tensor_mul(m2[:], k1, kB[64:128, cs])
            nc.vector.tensor_sub(kTn[0:64, cs], m1[:], m2[:])
            nc.vector.tensor_mul(m1[:], k1, kC[64:128, cs])
            nc.vector.tensor_mul(m2[:], k0, kD[0:64, cs])
            nc.vector.tensor_add(kTn[64:128, cs], m1[:], m2[:])

        k_rope(0)

        # =========== v pass with ksum / bc-p1 insertions ===========
        ksum = {c: ps_sm.tile([2, 512], F32, tag="ss", name="ks") for c in range(2)}
        bps1 = {}

        def ins_ksum(c):
            def f():
                nc.tensor.matmul(ksum[c][0:1, :], onesb,
                                 sqk[:, c * 512:(c + 1) * 512],
                                 start=True, stop=True)
            return f

        def ins_bc1():
            for c in range(2):
                bp = ps_bps.tile([128, 512], F32, tag="bps", name="bps")
                nc.tensor.matmul(bp[:], bcw[:], rstd1[c][:], start=True, stop=True)
                bps1[c] = bp

        units = []           # tt-major: per t-tile, all 4 q-heads
        for tt in range(NT):
            if not plan[tt]:
                continue
            for g in range(G):
                units.append((g, tt, tt // 4))
        udata = {}

        def emit_lg_exp_mask(u, lg_pool=None):
            g, tt, half = u
            ents = plan[tt]
            n_e = len(ents)
            w = n_e * 128
            lg = (lg_pool or ps_lg).tile([128, 512], F32, tag="proj" if lg_pool else "lg", name="lg")
            for ei, (si, kind, mi) in enumerate(ents):
                nc.tensor.matmul(lg[:, ei * 128:(ei + 1) * 128],
                                 kTn[:, si * 128:(si + 1) * 128],
                                 qh[g][:, tt * 128:(tt + 1) * 128],
                                 start=True, stop=True)
            P = sbP.tile([128, 512], BF16, tag="P", name="P")
            nc.scalar.activation(P[:, :w], lg[:, :w], EXPF, scale=1.0)
            spans = []
            for ei, (si, kind, mi) in enumerate(ents):
                if kind != "partial":
                    continue
                if spans and spans[-1][1] == ei:
                    spans[-1][1] = ei + 1
                else:
                    spans.append([ei, ei + 1, mi])
            for e0, e1, mi0 in spans:
                nc.vector.tensor_mul(
                    P[:, e0 * 128:e1 * 128], P[:, e0 * 128:e1 * 128],
                    mpk[:, mi0 * 128:(mi0 + (e1 - e0)) * 128])
            udata[u] = (P, n_e)

        _pre_units = units[:2]
        _pre_done = list(_pre_units)

        # v chunk 0 first: its psums stop early so V s-tiles 0..3 are
        # transposed + copied while v chunk 1 still projects
        pv0 = {}
        for di in range(ND):
            if di == 6:
                ins_ksum(0)()
            if di == 10:
                ins_bc1()
            if di == 9:
                ins_ksum(1)()
            if di == 0:
                pv0[0] = psproj.tile([128, 512], F32, tag="proj", name="proj")
            nc.tensor.matmul(pv0[0][:], wkv[:, D + di * 128:D + (di + 1) * 128],
                             xt[:, di * T: di * T + 512],
                             start=(di == 0), stop=(di == ND - 1))
        pv = {(D, 0): pv0[0]}

        # Act: sqrt ksum (SCALE*rstd_k = 1/sqrt(ssq + H*eps))
        kst = {}
        for c in range(2):
            st = sbr.tile([1, 512], F32, tag="kst", name="kst")
            nc.scalar.activation(st[:], ksum[c][0:1, :], SQRTF,
                                 bias=cst[0:1, 0:1], scale=1.0)
            kst[c] = st
        # DVE: per-chunk recip -> Pool broadcast -> fold; chunk-0 chain runs
        # before k_rope(1) so the first attention t-tiles unblock early
        def k_fold(c):
            cs = slice(c * 512, (c + 1) * 512)
            nc.vector.reciprocal(rk[0:1, cs], kst[c][:])
            nc.gpsimd.partition_broadcast(rkb[:, cs], rk[0:1, cs])
            nc.vector.tensor_mul(kTn[:, cs], kTn[:, cs], rkb[:, cs])

        k_fold(0)
        k_rope(1)
        k_fold(1)

        # dummy exp: forces the exp act-table load into this idle window,
        # before the first real attention exp
        dmy = sbr.tile([1, 2], F32, tag="dmy", name="dmy")
        nc.scalar.activation(dmy[:], kst[1][0:1, 0:2], EXPF, scale=1.0)

        # Act: bps p1 copies ; DVE (deferred into att loop): qh p1 mults
        bsb1 = {}
        for c in range(2):
            bsb = sbb.tile([128, 512], BF16, tag="bsb", name="bsb")
            nc.scalar.copy(bsb[:], bps1[c][:])
            bsb1[c] = bsb

        def qh_p1_mults(c):
            cs = slice(c * 512, (c + 1) * 512)
            ra, rb = rr1[c]
            bsb = bsb1[c]
            nc.vector.tensor_mul(qh[2][0:64, cs], ra[0:64, :], bsb[0:64, :])
            nc.vector.tensor_mul(qh[2][64:128, cs], rb[0:64, :], bsb[0:64, :])
            nc.vector.tensor_mul(qh[3][0:64, cs], ra[64:128, :], bsb[64:128, :])
            nc.vector.tensor_mul(qh[3][64:128, cs], rb[64:128, :], bsb[64:128, :])

        # Act: vt c0 copy ; PE: transpose + V copies for s-tiles 0..3
        nc.scalar.copy(vt_sb[:, 0:512], pv[(D, 0)][:])

        def vt_group(j0, j1):
            for j in range(j0, j1):
                vp = ps_bps.tile([128, 128], BF16, tag="vtpe", name="vtpe")
                nc.tensor.transpose(vp[:], vt_sb[:, j * 128:(j + 1) * 128], iden[:])
                nc.scalar.copy(V[:, j * 128:(j + 1) * 128], vp[:])

        # v chunk 1 pass (Vt c0 transposes + early att logit units overlap)
        pv1 = psproj.tile([128, 512], F32, tag="proj", name="proj")
        for di in range(ND):
            if di == 4:
                vt_group(0, 2)
            if di == 8:
                vt_group(2, 4)
            if di >= 8 and _pre_units:
                emit_lg_exp_mask(_pre_units.pop(0), lg_pool=psproj)
            nc.tensor.matmul(pv1[:], wkv[:, D + di * 128:D + (di + 1) * 128],
                             xt[:, di * T + 512: di * T + 1024],
                             start=(di == 0), stop=(di == ND - 1))
        nc.scalar.copy(vt_sb[:, 512:1024], pv1[:])

        # =========== phase 2: attention + out projection ===========
        es1.close()   # free phase-1 SBUF + PSUM
        sbrec = pool(name="sb_rec", bufs=4)        # quad reciprocals
        sbbc = pool(name="sb_bcs", bufs=4)         # quad broadcasts
        obp = pool(name="sb_ob", bufs=3)           # output staging
        ps_lg = pool(name="ps_lg", bufs=2, space="PSUM")
        ps_qkv = pool(name="ps_qkv", bufs=2, space="PSUM")
        ps_den = pool(name="ps_den", bufs=1, space="PSUM")
        ps_op = pool(name="ps_op", bufs=3, space="PSUM")

        def vt_group2(j0, j1):
            for j in range(j0, j1):
                vp = ps_op.tile([128, 128], BF16, tag="op", name="vtp")
                nc.tensor.transpose(vp[:], vt_sb[:, j * 128:(j + 1) * 128], iden[:])
                nc.scalar.copy(V[:, j * 128:(j + 1) * 128], vp[:])


        quad_ps = {}         # tt -> (den_ps, qkv_ps, n_done)

        udata = {}


        def emit_den_qkv(u):
            g, tt, half = u
            qk = quad_ps.get(tt)
            if qk is None:
                den = ps_den.tile([1, 512], F32, tag="den", name="den")
                qkv = ps_qkv.tile([128, 512], F32, tag="qkv", name="qkv")
                qk = quad_ps[tt] = [den, qkv, 0]
            den, qkv, _ = qk
            P, n_e = udata.pop(u)
            ents = plan[tt]
            gs = slice(g * 128, (g + 1) * 128)
            for ei, (si, kind, mi) in enumerate(ents):
                nc.tensor.matmul(den[0:1, gs], onesb,
                                 P[:, ei * 128:(ei + 1) * 128],
                                 start=(ei == 0), stop=(ei == n_e - 1))
            for ei, (si, kind, mi) in enumerate(ents):
                nc.tensor.matmul(qkv[:, gs],
                                 V[:, si * 128:(si + 1) * 128],
                                 P[:, ei * 128:(ei + 1) * 128],
                                 start=(ei == 0), stop=(ei == n_e - 1))
            qk[2] += 1
            if qk[2] == G:
                rec = sbrec.tile([1, 512], F32, tag="rec", name="rec")
                nc.vector.reciprocal(rec[:], den[0:1, :])
                bcs = sbbc.tile([128, 512], F32, tag="bcs", name="bcs")
                nc.gpsimd.partition_broadcast(bcs[:], rec[0:1, :])
                ts_ = slice(tt * 128, (tt + 1) * 128)
                for gg in range(G):
                    ggs = slice(gg * 128, (gg + 1) * 128)
                    nc.vector.tensor_mul(qkvh[gg][:, ts_], qkv[:, ggs],
                                         bcs[:, ggs])
                del quad_ps[tt]

        obs = {}

        def emit_outproj(tt, dc, w512=512):
            op = ps_op.tile([128, 512], F32, tag="op", name="op")
            for g in range(G):
                nc.tensor.matmul(op[:, 0:w512],
                                 qkvh[g][:, tt * 128:(tt + 1) * 128],
                                 wo[:, g * D + dc * 512: g * D + dc * 512 + w512],
                                 start=(g == 0), stop=(g == G - 1))
            ob = obs.get(tt)
            if ob is None:
                ob = obs[tt] = obp.tile([128, D], BF16, tag="ob", name="ob")
            if dc % 2 == 0:
                nc.vector.tensor_copy(ob[:, dc * 512:(dc + 1) * 512], op[:])
            else:
                nc.scalar.copy(ob[:, dc * 512:(dc + 1) * 512], op[:])
            if tt >= 4:
                q = (nc.sync, nc.scalar, nc.gpsimd, nc.sync)[dc]
                q.dma_start(
                    out_d[tt * 128:(tt + 1) * 128, dc * 512:(dc + 1) * 512],
                    ob[:, dc * 512:(dc + 1) * 512])
                if dc == 3:
                    del obs[tt]
            elif dc == 3:
                nc.sync.dma_start(out_d[tt * 128:(tt + 1) * 128, :], ob[:])
                del obs[tt]

        # --- attention half 0, software-pipelined depth 2 ---
        h0_units = [u for u in units if u[2] == 0]
        h1_units = [u for u in units if u[2] == 1]

        LAG = 3
        seq = []
        op_done = 0
        op_ready = []        # op units whose qkvh tt-quad has closed
        for i, u in enumerate(units):
            if u not in _pre_done:
                seq.append(("lg", u))
            if i == 1:
                seq.append(("vt", (4, 6)))
            if i == 2:
                seq.append(("vt", (6, 8)))
                seq.append(("qhp1", 0))
            if i == 5:
                seq.append(("qhp1", 1))
            if i >= LAG:
                v = units[i - LAG]
                seq.append(("dq", v))
                if v[0] == G - 1:           # tt quad closed -> ops ready
                    for dc in range(4):
                        op_ready.append((v[1], dc))
            # drain ready op units, keeping a small age buffer so the
            # tt-quad normalize chain has time to complete
            for _ in range(1):
                if op_done < len(op_ready) - 4:
                    seq.append(("op", op_ready[op_done]))
                    op_done += 1
        for v in units[-LAG:]:
            seq.append(("dq", v))
            if v[0] == G - 1:
                for dc in range(4):
                    op_ready.append((v[1], dc))
        for o in op_ready[op_done:]:
            seq.append(("op", o))

        for kind, arg in seq:
            if kind == "lg":
                emit_lg_exp_mask(arg)
            elif kind == "dq":
                emit_den_qkv(arg)
            elif kind == "vt":
                vt_group2(*arg)
            elif kind == "qhp1":
                qh_p1_mults(arg)
            elif kind == "op":
                emit_outproj(*arg)

    nc.finalize()
    return nc


_CACHE = {}


def kernel(x, segment_ids, Wq, Wk, Wv, Wo, q_scale, k_scale):
    global LAST_RESULTS
    import os
    import ml_dtypes

    bf = ml_dtypes.bfloat16
    x = np.asarray(x, np.float32)
    seg = np.asarray(segment_ids)
    Wq = np.asarray(Wq, np.float32)
    Wk = np.asarray(Wk, np.float32)
    Wv = np.asarray(Wv, np.float32)
    Wo = np.asarray(Wo, np.float32)
    q_scale = np.asarray(q_scale, np.float64)
    k_scale = np.asarray(k_scale, np.float64)

    plan, masks = _classify([seg[b] for b in range(B)])
    key = repr(plan)
    if key not in _CACHE:
        _CACHE[key] = _build_nc(plan, masks[0].shape[0])
    nc = _CACHE[key]

    half = H // 2
    timescale = ROPE_BASE ** (2.0 * np.arange(half, dtype=np.float64) / H)
    qs_lo = np.tile(q_scale[:64], 2)[:, None]
    qs_hi = np.tile(q_scale[64:], 2)[:, None]
    ks_lo = np.tile(k_scale[:64], 2)[:, None]
    ks_hi = np.tile(k_scale[64:], 2)[:, None]
    rtq_b, rtk_b = [], []
    for b in range(B):
        pos = _positions(seg[b])
        sinus = pos[None, :] / timescale[:, None]        # [64, T]
        sd = np.vstack([np.sin(sinus)] * 2)              # [128, T]
        cd = np.vstack([np.cos(sinus)] * 2)
        rtq_b.append(np.hstack([qs_lo * cd, qs_hi * sd, qs_hi * cd, qs_lo * sd]
                               ).astype(bf))
        rtk_b.append(np.hstack([ks_lo * cd, ks_hi * sd, ks_hi * cd, ks_lo * sd]
                               ).astype(bf))

    cstb = np.zeros((128, 4), np.float32)
    cstb[0:64, 0] = 1.0
    cstb[64:128, 1] = 1.0
    cstb[:, 2] = 1.0
    cst = np.zeros((128, 2), np.float32)
    cst[:, 0] = H * EPS
    cst[:, 1] = EPS
    bcw = np.zeros((2, 128), np.float32)
    bcw[0, 0:64] = 1.0
    bcw[1, 64:128] = 1.0
    iden = np.eye(128, dtype=np.float32)

    in_maps = []
    for core in range(8):
        b, kv = core // K, core % K
        xt = np.ascontiguousarray(
            x[b].T.reshape(ND, 128, T).transpose(1, 0, 2).reshape(128, ND * T))

        def qfeat(w, cols):
            # [D, 128] -> [128(d_lo), ND*128] with w[p, di*128+j] = W[di*128+p, cols[j]]
            sub = w[:, cols]                             # [D, 128]
            return sub.reshape(ND, 128, 128).transpose(1, 0, 2).reshape(128, D)

        base = kv * 4 * H
        f_cols = []
        for pair in range(2):      # (f0,f2) then (f1,f3)
            for hv in range(2):
                cols = np.concatenate([
                    np.arange(base + (2 * g4 + pair) * H + hv * 64,
                              base + (2 * g4 + pair) * H + hv * 64 + 64)
                    for g4 in range(2)])
                f_cols.append(cols)
        # heads order per pair: pair0 -> heads (0,1)?  cols above pick heads
        # (pair + 2*g4): pair0 -> heads 0,2 ... fix: want pair0 = heads 0,1.
        f_cols = []
        for pair, heads in [(0, (0, 1)), (1, (2, 3))]:
            for hv in range(2):
                cols = np.concatenate([
                    np.arange(base + g4 * H + hv * 64,
                              base + g4 * H + hv * 64 + 64) for g4 in heads])
                f_cols.append(cols)
        wq0 = np.hstack([qfeat(Wq, f_cols[0]), qfeat(Wq, f_cols[1])]).astype(bf)
        wq1 = np.hstack([qfeat(Wq, f_cols[2]), qfeat(Wq, f_cols[3])]).astype(bf)
        kcols = np.arange(kv * H, (kv + 1) * H)
        wkv = np.hstack([qfeat(Wk, kcols), qfeat(Wv, kcols)]).astype(bf)
        wo_t = np.ascontiguousarray(
            Wo[kv * 512:(kv + 1) * 512].reshape(G, 128, D)
            .transpose(1, 0, 2).reshape(128, G * D)).astype(bf)
        nm = max(masks[b].shape[0], 1)
        mpk = np.ascontiguousarray(
            masks[b].transpose(1, 0, 2).reshape(128, nm * 128)).astype(bf)

        in_maps.append({
            "xt": xt.astype(bf), "wq0": wq0, "wq1": wq1, "wkv": wkv,
            "wo": wo_t, "rtq": rtq_b[b], "rtk": rtk_b[b], "mpk": mpk,
            "iden": iden.astype(bf), "cstb": cstb.astype(bf),
            "cst": cst, "bcw": bcw,
        })

    do_trace = os.environ.get("BASS_TRACE") == "1"
    res = run_bass_kernel_spmd(
        nc, in_maps, core_ids=list(range(8)), trace=do_trace)
    LAST_RESULTS = res

    out = np.zeros((B, T, D), np.float32)
    for core in range(8):
        out[core // K] += res.results[core]["out"].astype(np.float32)
    return out


# revision 6
# speedup vs baseline: 1.0298x; 1.0007x over previous
"""Trainium2 Bass kernel v2 for segment-causal GQA attention.

Sharding: 8 cores = batch (2) x kv-head (4), as baseline. All matmul
operands bf16 (1 cyc/row at any free size), halving DMA and enabling
128-wide attention t-tiles. RMS scales folded into host rope tables;
SCALE*rstd_k folded into kTn columns so exp batches over s-tiles.
Attention: per (g, t-tile) unit = n_e logit matmuls into one psum run,
one exp, one mask mult, n_e den + n_e qkv matmuls; den/qkv batched
per (g, 4-tt quad) for one reciprocal + broadcast + normalize.
"""

import sys

sys.path.insert(0, "/opt/trn_rl_repo")

import numpy as np

import concourse.bacc as bacc
import concourse.bass as bass  # noqa: F401
import concourse.tile as tile
from concourse import mybir
from concourse.bass_utils import run_bass_kernel_spmd

B, T, D = 2, 1024, 2048
N, K, H = 16, 4, 128
G = N // K
EPS = 1e-6
ROPE_BASE = 10000.0
ND = D // 128        # 16 d-tiles
NS = T // 128        # 8 s-tiles
NT = T // 128        # 8 t-tiles
F32 = mybir.dt.float32
F32R = mybir.dt.float32r
BF16 = mybir.dt.bfloat16
EXPF = mybir.ActivationFunctionType.Exp
SQRTF = mybir.ActivationFunctionType.Sqrt

LAST_RESULTS = None  # test harness reads exec_time_ns from here


def _positions(seg):
    t = seg.shape[0]
    idx = np.arange(t, dtype=np.int64)
    is_start = np.concatenate([[True], seg[1:] != seg[:-1]])
    seg_start = np.maximum.accumulate(np.where(is_start, idx, 0))
    return (idx - seg_start).astype(np.float64)


def _classify(seg_rows):
    """Union tile classification over batches at 128x128 granularity.

    Returns (plan, masks_per_batch): plan[tt] = list of (si, kind, mask_idx);
    masks_per_batch[b] = float32 [max(n_masks,1), 128, 128] of 0/1.
    """
    idx = np.arange(T)
    valids = []
    for b in range(len(seg_rows)):
        seg = seg_rows[b]
        valids.append((seg[:, None] == seg[None, :]) & (idx[:, None] <= idx[None, :]))
    plan = []
    mask_list = [[] for _ in range(len(seg_rows))]
    n_masks = 0
    for tt in range(NT):
        t0 = tt * 128
        entries = []
        for si in range(NS):
            s0 = si * 128
            subs = [v[s0:s0 + 128, t0:t0 + 128] for v in valids]
            if not any(s.any() for s in subs):
                continue
            if all(s.all() for s in subs):
                entries.append((si, "full", -1))
            else:
                for b in range(len(seg_rows)):
                    mask_list[b].append(subs[b].astype(np.float32))
                entries.append((si, "partial", n_masks))
                n_masks += 1
        plan.append(entries)
    masks = []
    for b in range(len(seg_rows)):
        if n_masks:
            masks.append(np.ascontiguousarray(np.stack(mask_list[b]), np.float32))
        else:
            masks.append(np.zeros((1, 128, 128), np.float32))
    return plan, masks


def _build_nc(plan, n_masks):
    from contextlib import ExitStack

    nc = bacc.Bacc(None, target_bir_lowering=False, debug=False)
    MULT = mybir.AluOpType.mult

    xt_d = nc.dram_tensor("xt", [128, ND * T], BF16, kind="ExternalInput")
    wq0_d = nc.dram_tensor("wq0", [128, 2 * D], BF16, kind="ExternalInput")
    wq1_d = nc.dram_tensor("wq1", [128, 2 * D], BF16, kind="ExternalInput")
    wkv_d = nc.dram_tensor("wkv", [128, 2 * D], BF16, kind="ExternalInput")
    wo_d = nc.dram_tensor("wo", [128, G * D], BF16, kind="ExternalInput")
    rtq_d = nc.dram_tensor("rtq", [128, 4 * T], BF16, kind="ExternalInput")
    rtk_d = nc.dram_tensor("rtk", [128, 4 * T], BF16, kind="ExternalInput")
    nm = max(n_masks, 1)
    mpk_d = nc.dram_tensor("mpk", [128, nm * 128], BF16, kind="ExternalInput")
    iden_d = nc.dram_tensor("iden", [128, 128], BF16, kind="ExternalInput")
    cstb_d = nc.dram_tensor("cstb", [128, 4], BF16, kind="ExternalInput")
    cst_d = nc.dram_tensor("cst", [128, 2], F32, kind="ExternalInput")
    bc_d = nc.dram_tensor("bcw", [2, 128], F32R, kind="ExternalInput")
    out_d = nc.dram_tensor("out", [T, D], BF16, kind="ExternalOutput")

    es = ExitStack()
    with es:
        es.enter_context(nc.allow_low_precision("bf16 kernel"))
        tc = es.enter_context(tile.TileContext(nc))
        pool = lambda *a, **k: es.enter_context(tc.tile_pool(*a, **k))
        pp = pool(name="persist", bufs=1)

        # ---------------- persistent SBUF tiles ----------------
        xt = pp.tile([128, ND * T], BF16, tag="xt", name="xt")
        wq0 = pp.tile([128, 2 * D], BF16, tag="wq0", name="wq0")
        wq1 = pp.tile([128, 2 * D], BF16, tag="wq1", name="wq1")
        wkv = pp.tile([128, 2 * D], BF16, tag="wkv", name="wkv")
        wo = pp.tile([128, G * D], BF16, tag="wo", name="wo")
        rtq = pp.tile([128, 4 * T], BF16, tag="rtq", name="rtq")
        rtk = pp.tile([128, 4 * T], BF16, tag="rtk", name="rtk")
        mpk = pp.tile([128, nm * 128], BF16, tag="mpk", name="mpk")
        iden = pp.tile([128, 128], BF16, tag="iden", name="iden")
        cstb = pp.tile([128, 4], BF16, tag="cstb", name="cstb")
        cst = pp.tile([128, 2], F32, tag="cst", name="cst")
        bcw = pp.tile([2, 128], F32R, tag="bcw", name="bcw")
        qh = [pp.tile([128, T], BF16, tag=f"qh{g}", name=f"qh{g}") for g in range(G)]
        kTn = pp.tile([128, T], BF16, tag="kTn", name="kTn")
        V = pp.tile([128, T], BF16, tag="V", name="V")
        vt_sb = pp.tile([128, T], BF16, tag="vt", name="vt")
        sqk = pp.tile([128, T], BF16, tag="sqk", name="sqk")
        qkvh = [pp.tile([128, T], BF16, tag=f"qkvh{g}", name=f"qkvh{g}")
                for g in range(G)]
        rk = pp.tile([1, T], F32, tag="rk", name="rk")
        rkb = pp.tile([128, T], F32, tag="rkb", name="rkb")

        onesb = cstb[:, 2:3]          # bf16 ones column (den / ksum lhsT)

        # ---------------- DMAs (sync queue, priority order) ----------------
        def xchunk(ci):
            sl = slice(ci * 2 * T, (ci + 1) * 2 * T)
            nc.sync.dma_start(xt[:, sl], xt_d[:, sl])

        nc.sync.dma_start(xt[:, 0:T], xt_d[:, 0:T])                # di0
        nc.sync.dma_start(wq0[:, 0:512], wq0_d[:, 0:512])          # f0 di0-3
        nc.sync.dma_start(xt[:, T:2 * T], xt_d[:, T:2 * T])        # di1
        nc.sync.dma_start(wq0[:, D:D + 512], wq0_d[:, D:D + 512])  # f2 di0-3
        xchunk(1)
        nc.sync.dma_start(wq0[:, 512:D], wq0_d[:, 512:D])          # f0 di4-15
        xchunk(2)
        nc.sync.dma_start(wq0[:, D + 512:2 * D], wq0_d[:, D + 512:2 * D])
        xchunk(3)
        for t_, d_ in [(cstb, cstb_d), (cst, cst_d), (bcw, bc_d),
                       (iden, iden_d)]:
            nc.sync.dma_start(t_[:], d_[:])
        for ci in range(4, 8):
            xchunk(ci)
        nc.sync.dma_start(wq1[:], wq1_d[:])
        nc.sync.dma_start(rtq[:], rtq_d[:])
        nc.sync.dma_start(wkv[:], wkv_d[:])
        nc.sync.dma_start(rtk[:], rtk_d[:])
        nc.sync.dma_start(wo[:], wo_d[:])
        nc.sync.dma_start(mpk[:], mpk_d[:])

        # ---------------- stream pools ----------------
        sbra = pool(name="sb_ra", bufs=5)          # rope outputs ra/rb
        sbb = pool(name="sb_bps", bufs=3)          # bps broadcast sbuf
        sbP = pool(name="sb_P", bufs=8)            # attention P tiles
        es1 = ExitStack()
        pool1 = lambda *a, **k: es1.enter_context(tc.tile_pool(*a, **k))
        sbpc = pool1(name="sb_pc", bufs=8)         # psum->sbuf proj copies
        sbm = pool1(name="sb_m", bufs=2)           # rope temporaries
        sbq = pool1(name="sb_sq", bufs=6)          # square tiles
        sbr = pool1(name="sb_rstd", bufs=2)        # rstd / stmp tiles

        psproj = pool1(name="ps_proj", bufs=4, space="PSUM")
        ps_sm = pool1(name="ps_small", bufs=2, space="PSUM")
        ps_bps = pool1(name="ps_bps", bufs=1, space="PSUM")

        # PE p-state warmup: matmuls on an uninitialized scratch tile (values
        # never read) while the input DMAs stream in
        wsc = sbm.tile([128, 128], BF16, tag="m1", name="wscratch")
        nc.gpsimd.memset(wsc[:], 0.0)
        warm = psproj.tile([128, 512], F32, tag="proj", name="warm")
        for _ in range(18):
            nc.tensor.matmul(warm[:, 0:128], wsc[:], wsc[:],
                             start=True, stop=True)
        wdmy = sbr.tile([1, 2], F32, tag="dmy", name="wdmy")
        nc.scalar.copy(wdmy[:], warm[0:1, 0:2])

        def proj_pass(w, feats, inserts=None, fillers=0, pre_pss=None):
            """d-outer pass over `feats` = list of (col_off, psum pair).
            inserts: {di: fn} PE-stream injections. fillers: p-state keepalive
            matmuls per di boundary while the xt stream is still arriving."""
            pss = dict(pre_pss) if pre_pss else {}
            for fo, _ in feats:
                for c in range(2):
                    if (fo, c) not in pss:
                        pss[(fo, c)] = psproj.tile([128, 512], F32, tag="proj",
                                                   name="proj")
            for di in range(ND):
                if inserts and di in inserts:
                    inserts[di]()
                if fillers and di < 8:
                    for _ in range(fillers):
                        nc.tensor.matmul(warm[:, 128:256], wsc[:], wsc[:],
                                         start=True, stop=True)
                for fo, _ in feats:
                    for c in range(2):
                        nc.tensor.matmul(
                            pss[(fo, c)][:],
                            w[:, fo + di * 128: fo + (di + 1) * 128],
                            xt[:, di * T + c * 512: di * T + (c + 1) * 512],
                            start=(di == 0), stop=(di == ND - 1))
            return pss

        def rope_q(pca, pcb, ra, rb, cs):
            m1 = sbm.tile([128, 512], BF16, tag="m1", name="m1")
            m2 = sbm.tile([128, 512], BF16, tag="m2", name="m2")
            qA, qB = rtq[:, 0 * T:1 * T], rtq[:, 1 * T:2 * T]
            qC, qD = rtq[:, 2 * T:3 * T], rtq[:, 3 * T:4 * T]
            nc.vector.tensor_mul(m1[:], pca[:], qA[:, cs])
            nc.vector.tensor_mul(m2[:], pcb[:], qB[:, cs])
            nc.vector.tensor_sub(ra[:], m1[:], m2[:])
            nc.vector.tensor_mul(m1[:], pcb[:], qC[:, cs])
            nc.vector.tensor_mul(m2[:], pca[:], qD[:, cs])
            nc.vector.tensor_add(rb[:], m1[:], m2[:])

        # =========== phase 1: q0 pass ===========
        pq0 = proj_pass(wq0, [(0, None), (D, None)])
        # Act: psum -> sbuf bf16 copies + squares
        pcs0, sqs0 = {}, {}
        for c in range(2):
            for fi, fo in enumerate((0, D)):
                pc = sbpc.tile([128, 512], BF16, tag="pc", name="pc")
                nc.scalar.copy(pc[:], pq0[(fo, c)][:])
                pcs0[(fi, c)] = pc
        for c in range(2):
            for fi in range(2):
                sq = sbq.tile([128, 512], BF16, tag="sq", name="sq")
                nc.scalar.square(sq[:], pcs0[(fi, c)][:])
                sqs0[(fi, c)] = sq
        # DVE: rope p0
        rr0 = {}
        for c in range(2):
            cs = slice(c * 512, (c + 1) * 512)
            ra = sbra.tile([128, 512], BF16, tag="ra", name="ra")
            rb = sbra.tile([128, 512], BF16, tag="rb", name="rb")
            rope_q(pcs0[(0, c)], pcs0[(1, c)], ra, rb, cs)
            rr0[c] = (ra, rb)

        # =========== q1 pass with rstd-p0 insertions ===========
        ss0 = {c: ps_sm.tile([2, 512], F32, tag="ss", name="ss") for c in range(2)}

        def ins_ssq0():
            for c in range(2):
                for fi in range(2):
                    nc.tensor.matmul(ss0[c][:], cstb[:, 0:2], sqs0[(fi, c)][:],
                                     start=(fi == 0), stop=(fi == 1))

        rstd0 = {}
        for c in range(2):
            rstd0[c] = sbr.tile([2, 512], F32R, tag="rstd", name="rstd")

        pq1 = proj_pass(wq1, [(0, None), (D, None)],
                        inserts={10: ins_ssq0},
                        pre_pss={(0, 0): ps_bps.tile([128, 512], F32,
                                                     tag="bps", name="q1pre")})

        # Act: sqrt ss p0 ; DVE: recip -> rstd0
        for c in range(2):
            st = sbr.tile([2, 512], F32, tag="stmp", name="stmp")
            nc.scalar.activation(st[:], ss0[c][:], SQRTF,
                                 bias=cst[0:2, 1:2], scale=float(1.0 / H))
            nc.vector.reciprocal(rstd0[c][:], st[:])
        # PE: bc p0 broadcast matmuls (after rstd0 writers are emitted)
        bps0 = {}
        for c in range(2):
            bp = ps_bps.tile([128, 512], F32, tag="bps", name="bps")
            nc.tensor.matmul(bp[:], bcw[:], rstd0[c][:], start=True, stop=True)
            bps0[c] = bp
        # Act: bps copies ; DVE: qh p0 mults
        for c in range(2):
            bsb = sbb.tile([128, 512], BF16, tag="bsb", name="bsb")
            nc.scalar.copy(bsb[:], bps0[c][:])
            cs = slice(c * 512, (c + 1) * 512)
            ra, rb = rr0[c]
            nc.vector.tensor_mul(qh[0][0:64, cs], ra[0:64, :], bsb[0:64, :])
            nc.vector.tensor_mul(qh[0][64:128, cs], rb[0:64, :], bsb[0:64, :])
            nc.vector.tensor_mul(qh[1][0:64, cs], ra[64:128, :], bsb[64:128, :])
            nc.vector.tensor_mul(qh[1][64:128, cs], rb[64:128, :], bsb[64:128, :])

        # Act: q1 copies + squares ; DVE: rope p1
        pcs1, sqs1 = {}, {}
        for c in range(2):
            for fi, fo in enumerate((0, D)):
                pc = sbpc.tile([128, 512], BF16, tag="pc", name="pc")
                nc.scalar.copy(pc[:], pq1[(fo, c)][:])
                pcs1[(fi, c)] = pc
        for c in range(2):
            for fi in range(2):
                sq = sbq.tile([128, 512], BF16, tag="sq", name="sq")
                nc.scalar.square(sq[:], pcs1[(fi, c)][:])
                sqs1[(fi, c)] = sq
        rr1 = {}
        for c in range(2):
            cs = slice(c * 512, (c + 1) * 512)
            ra = sbra.tile([128, 512], BF16, tag="ra", name="ra")
            rb = sbra.tile([128, 512], BF16, tag="rb", name="rb")
            rope_q(pcs1[(0, c)], pcs1[(1, c)], ra, rb, cs)
            rr1[c] = (ra, rb)

        # =========== k pass with ssq-p1 insertion ===========
        ss1 = {c: ps_sm.tile([2, 512], F32, tag="ss", name="ss") for c in range(2)}

        def ins_ssq1():
            for c in range(2):
                for fi in range(2):
                    nc.tensor.matmul(ss1[c][:], cstb[:, 0:2], sqs1[(fi, c)][:],
                                     start=(fi == 0), stop=(fi == 1))

        pk = proj_pass(wkv, [(0, None)], inserts={12: ins_ssq1})

        rstd1 = {c: sbr.tile([2, 512], F32R, tag="rstd", name="rstd")
                 for c in range(2)}
        for c in range(2):
            st = sbr.tile([2, 512], F32, tag="stmp", name="stmp")
            nc.scalar.activation(st[:], ss1[c][:], SQRTF,
                                 bias=cst[0:2, 1:2], scale=float(1.0 / H))
            nc.vector.reciprocal(rstd1[c][:], st[:])

        # Act: pck copies + sqk squares ; DVE: k rope
        pck = {}
        for c in range(2):
            pc = sbpc.tile([128, 512], BF16, tag="pc", name="pc")
            nc.scalar.copy(pc[:], pk[(0, c)][:])
            pck[c] = pc
            cs = slice(c * 512, (c + 1) * 512)
            nc.scalar.square(sqk[:, cs], pc[:])
        kA, kB = rtk[:, 0 * T:1 * T], rtk[:, 1 * T:2 * T]
        kC, kD = rtk[:, 2 * T:3 * T], rtk[:, 3 * T:4 * T]

        def k_rope(c):
            cs = slice(c * 512, (c + 1) * 512)
            m1 = sbm.tile([64, 512], BF16, tag="km1", name="km1")
            m2 = sbm.tile([64, 512], BF16, tag="km2", name="km2")
            k0, k1 = pck[c][0:64, :], pck[c][64:128, :]
            nc.vector.tensor_mul(m1[:], k0, kA[0:64, cs])
            nc.vector.tensor_mul(m2[:], k1, kB[64:128, cs])
            nc.vector.tensor_sub(kTn[0:64, cs], m1[:], m2[:])
            nc.vector.tensor_mul(m1[:], k1, kC[64:128, cs])
            nc.vector.tensor_mul(m2[:], k0, kD[0:64, cs])
            nc.vector.tensor_add(kTn[64:128, cs], m1[:], m2[:])

        k_rope(0)

        # =========== v pass with ksum / bc-p1 insertions ===========
        ksum = {c: ps_sm.tile([2, 512], F32, tag="ss", name="ks") for c in range(2)}
        bps1 = {}

        def ins_ksum(c):
            def f():
                nc.tensor.matmul(ksum[c][0:1, :], onesb,
                                 sqk[:, c * 512:(c + 1) * 512],
                                 start=True, stop=True)
            return f

        def ins_bc1():
            for c in range(2):
                bp = ps_bps.tile([128, 512], F32, tag="bps", name="bps")
                nc.tensor.matmul(bp[:], bcw[:], rstd1[c][:], start=True, stop=True)
                bps1[c] = bp

        units = []           # tt-major: per t-tile, all 4 q-heads
        for tt in range(NT):
            if not plan[tt]:
                continue
            for g in range(G):
                units.append((g, tt, tt // 4))
        udata = {}

        def emit_lg_exp_mask(u, lg_pool=None):
            g, tt, half = u
            ents = plan[tt]
            n_e = len(ents)
            w = n_e * 128
            lg = (lg_pool or ps_lg).tile([128, 512], F32, tag="proj" if lg_pool else "lg", name="lg")
            for ei, (si, kind, mi) in enumerate(ents):
                nc.tensor.matmul(lg[:, ei * 128:(ei + 1) * 128],
                                 kTn[:, si * 128:(si + 1) * 128],
                                 qh[g][:, tt * 128:(tt + 1) * 128],
                                 start=True, stop=True)
            P = sbP.tile([128, 512], BF16, tag="P", name="P")
            nc.scalar.activation(P[:, :w], lg[:, :w], EXPF, scale=1.0)
            spans = []
            for ei, (si, kind, mi) in enumerate(ents):
                if kind != "partial":
                    continue
                if spans and spans[-1][1] == ei:
                    spans[-1][1] = ei + 1
                else:
                    spans.append([ei, ei + 1, mi])
            for e0, e1, mi0 in spans:
                nc.vector.tensor_mul(
                    P[:, e0 * 128:e1 * 128], P[:, e0 * 128:e1 * 128],
                    mpk[:, mi0 * 128:(mi0 + (e1 - e0)) * 128])
            udata[u] = (P, n_e)

        _pre_units = units[:2]
        _pre_done = list(_pre_units)

        # v chunk 0 first: its psums stop early so V s-tiles 0..3 are
        # transposed + copied while v chunk 1 still projects
        pv0 = {}
        for di in range(ND):
            if di == 6:
                ins_ksum(0)()
            if di == 10:
                ins_bc1()
            if di == 9:
                ins_ksum(1)()
            if di == 0:
                pv0[0] = psproj.tile([128, 512], F32, tag="proj", name="proj")
            nc.tensor.matmul(pv0[0][:], wkv[:, D + di * 128:D + (di + 1) * 128],
                             xt[:, di * T: di * T + 512],
                             start=(di == 0), stop=(di == ND - 1))
        pv = {(D, 0): pv0[0]}

        # Act: sqrt ksum (SCALE*rstd_k = 1/sqrt(ssq + H*eps))
        kst = {}
        for c in range(2):
            st = sbr.tile([1, 512], F32, tag="kst", name="kst")
            nc.scalar.activation(st[:], ksum[c][0:1, :], SQRTF,
                                 bias=cst[0:1, 0:1], scale=1.0)
            kst[c] = st
        # DVE: per-chunk recip -> Pool broadcast -> fold; chunk-0 chain runs
        # before k_rope(1) so the first attention t-tiles unblock early
        def k_fold(c):
            cs = slice(c * 512, (c + 1) * 512)
            nc.vector.reciprocal(rk[0:1, cs], kst[c][:])
            nc.gpsimd.partition_broadcast(rkb[:, cs], rk[0:1, cs])
            nc.vector.tensor_mul(kTn[:, cs], kTn[:, cs], rkb[:, cs])

        # Act: bps p1 copies ; DVE: qh p1 mults hoisted into the k-fold
        # window so attention's g2/g3 units never wait on them
        bsb1 = {}
        for c in range(2):
            bsb = sbb.tile([128, 512], BF16, tag="bsb", name="bsb")
            nc.scalar.copy(bsb[:], bps1[c][:])
            bsb1[c] = bsb

        def qh_p1_mults(c):
            cs = slice(c * 512, (c + 1) * 512)
            ra, rb = rr1[c]
            bsb = bsb1[c]
            nc.vector.tensor_mul(qh[2][0:64, cs], ra[0:64, :], bsb[0:64, :])
            nc.vector.tensor_mul(qh[2][64:128, cs], rb[0:64, :], bsb[0:64, :])
            nc.vector.tensor_mul(qh[3][0:64, cs], ra[64:128, :], bsb[64:128, :])
            nc.vector.tensor_mul(qh[3][64:128, cs], rb[64:128, :], bsb[64:128, :])

        k_fold(0)
        qh_p1_mults(0)
        k_rope(1)
        k_fold(1)
        qh_p1_mults(1)

        # dummy exp: forces the exp act-table load into this idle window,
        # before the first real attention exp
        dmy = sbr.tile([1, 2], F32, tag="dmy", name="dmy")
        nc.scalar.activation(dmy[:], kst[1][0:1, 0:2], EXPF, scale=1.0)

        # Act: vt c0 copy ; PE: transpose + V copies for s-tiles 0..3
        nc.scalar.copy(vt_sb[:, 0:512], pv[(D, 0)][:])

        def vt_group(j0, j1):
            for j in range(j0, j1):
                vp = ps_bps.tile([128, 128], BF16, tag="vtpe", name="vtpe")
                nc.tensor.transpose(vp[:], vt_sb[:, j * 128:(j + 1) * 128], iden[:])
                nc.scalar.copy(V[:, j * 128:(j + 1) * 128], vp[:])

        # v chunk 1 pass (Vt c0 transposes + early att logit units overlap)
        pv1 = psproj.tile([128, 512], F32, tag="proj", name="proj")
        for di in range(ND):
            if di == 4:
                vt_group(0, 2)
            if di == 8:
                vt_group(2, 4)
            if di >= 8 and _pre_units:
                emit_lg_exp_mask(_pre_units.pop(0), lg_pool=psproj)
            nc.tensor.matmul(pv1[:], wkv[:, D + di * 128:D + (di + 1) * 128],
                             xt[:, di * T + 512: di * T + 1024],
                             start=(di == 0), stop=(di == ND - 1))
        nc.scalar.copy(vt_sb[:, 512:1024], pv1[:])

        # =========== phase 2: attention + out projection ===========
        es1.close()   # free phase-1 SBUF + PSUM
        sbrec = pool(name="sb_rec", bufs=4)        # quad reciprocals
        sbbc = pool(name="sb_bcs", bufs=4)         # quad broadcasts
        obp = pool(name="sb_ob", bufs=3)           # output staging
        ps_lg = pool(name="ps_lg", bufs=2, space="PSUM")
        ps_qkv = pool(name="ps_qkv", bufs=2, space="PSUM")
        ps_den = pool(name="ps_den", bufs=1, space="PSUM")
        ps_op = pool(name="ps_op", bufs=3, space="PSUM")

        def vt_group2(j0, j1):
            for j in range(j0, j1):
                vp = ps_op.tile([128, 128], BF16, tag="op", name="vtp")
                nc.tensor.transpose(vp[:], vt_sb[:, j * 128:(j + 1) * 128], iden[:])
                nc.scalar.copy(V[:, j * 128:(j + 1) * 128], vp[:])


        quad_ps = {}         # tt -> (den_ps, qkv_ps, n_done)

        udata = {}


        def emit_den_qkv(u):
            g, tt, half = u
            qk = quad_ps.get(tt)
            if qk is None:
                den = ps_den.tile([1, 512], F32, tag="den", name="den")
                qkv = ps_qkv.tile([128, 512], F32, tag="qkv", name="qkv")
                qk = quad_ps[tt] = [den, qkv, 0]
            den, qkv, _ = qk
            P, n_e = udata.pop(u)
            ents = plan[tt]
            gs = slice(g * 128, (g + 1) * 128)
            for ei, (si, kind, mi) in enumerate(ents):
                nc.tensor.matmul(den[0:1, gs], onesb,
                                 P[:, ei * 128:(ei + 1) * 128],
                                 start=(ei == 0), stop=(ei == n_e - 1))
            for ei, (si, kind, mi) in enumerate(ents):
                nc.tensor.matmul(qkv[:, gs],
                                 V[:, si * 128:(si + 1) * 128],
                                 P[:, ei * 128:(ei + 1) * 128],
                                 start=(ei == 0), stop=(ei == n_e - 1))
            qk[2] += 1
            if qk[2] == G:
                rec = sbrec.tile([1, 512], F32, tag="rec", name="rec")
                nc.vector.reciprocal(rec[:], den[0:1, :])
                bcs = sbbc.tile([128, 512], F32, tag="bcs", name="bcs")
                nc.gpsimd.partition_broadcast(bcs[:], rec[0:1, :])
                ts_ = slice(tt * 128, (tt + 1) * 128)
                for gg in range(G):
                    ggs = slice(gg * 128, (gg + 1) * 128)
                    nc.vector.tensor_mul(qkvh[gg][:, ts_], qkv[:, ggs],
                                         bcs[:, ggs])
                del quad_ps[tt]

        obs = {}

        def emit_outproj(tt, dc, w512=512):
            op = ps_op.tile([128, 512], F32, tag="op", name="op")
            for g in range(G):
                nc.tensor.matmul(op[:, 0:w512],
                                 qkvh[g][:, tt * 128:(tt + 1) * 128],
                                 wo[:, g * D + dc * 512: g * D + dc * 512 + w512],
                                 start=(g == 0), stop=(g == G - 1))
            ob = obs.get(tt)
            if ob is None:
                ob = obs[tt] = obp.tile([128, D], BF16, tag="ob", name="ob")
            if dc % 2 == 0:
                nc.vector.tensor_copy(ob[:, dc * 512:(dc + 1) * 512], op[:])
            else:
                nc.scalar.copy(ob[:, dc * 512:(dc + 1) * 512], op[:])
            if tt >= 4:
                q = (nc.sync, nc.scalar, nc.gpsimd, nc.sync)[dc]
                q.dma_start(
                    out_d[tt * 128:(tt + 1) * 128, dc * 512:(dc + 1) * 512],
                    ob[:, dc * 512:(dc + 1) * 512])
                if dc == 3:
                    del obs[tt]
            elif dc == 3:
                nc.sync.dma_start(out_d[tt * 128:(tt + 1) * 128, :], ob[:])
                del obs[tt]

        # --- attention half 0, software-pipelined depth 2 ---
        h0_units = [u for u in units if u[2] == 0]
        h1_units = [u for u in units if u[2] == 1]

        LAG = 4
        seq = []
        op_done = 0
        op_ready = []        # op units whose qkvh tt-quad has closed
        for i, u in enumerate(units):
            if u not in _pre_done:
                seq.append(("lg", u))
            if i == 1:
                seq.append(("vt", (4, 6)))
            if i == 2:
                seq.append(("vt", (6, 8)))
                seq.append(("qhp1", 0))
            if i == 5:
                seq.append(("qhp1", 1))
            if i >= LAG:
                v = units[i - LAG]
                seq.append(("dq", v))
                if v[0] == G - 1:           # tt quad closed -> ops ready
                    for dc in range(4):
                        op_ready.append((v[1], dc))
            # drain ready op units, keeping a small age buffer so the
            # tt-quad normalize chain has time to complete
            for _ in range(1):
                if op_done < len(op_ready) - 4:
                    seq.append(("op", op_ready[op_done]))
                    op_done += 1
        for v in units[-LAG:]:
            seq.append(("dq", v))
            if v[0] == G - 1:
                for dc in range(4):
                    op_ready.append((v[1], dc))
        for o in op_ready[op_done:]:
            seq.append(("op", o))

        for kind, arg in seq:
            if kind == "lg":
                emit_lg_exp_mask(arg)
            elif kind == "dq":
                emit_den_qkv(arg)
            elif kind == "vt":
                vt_group2(*arg)
            elif kind == "qhp1":
                qh_p1_mults(arg)
            elif kind == "op":
                emit_outproj(*arg)

    nc.finalize()
    return nc


_CACHE = {}


def kernel(x, segment_ids, Wq, Wk, Wv, Wo, q_scale, k_scale):
    global LAST_RESULTS
    import os
    import ml_dtypes

    bf = ml_dtypes.bfloat16
    x = np.asarray(x, np.float32)
    seg = np.asarray(segment_ids)
    Wq = np.asarray(Wq, np.float32)
    Wk = np.asarray(Wk, np.float32)
    Wv = np.asarray(Wv, np.float32)
    Wo = np.asarray(Wo, np.float32)
    q_scale = np.asarray(q_scale, np.float64)
    k_scale = np.asarray(k_scale, np.float64)

    plan, masks = _classify([seg[b] for b in range(B)])
    key = repr(plan)
    if key not in _CACHE:
        _CACHE[key] = _build_nc(plan, masks[0].shape[0])
    nc = _CACHE[key]

    half = H // 2
    timescale = ROPE_BASE ** (2.0 * np.arange(half, dtype=np.float64) / H)
    qs_lo = np.tile(q_scale[:64], 2)[:, None]
    qs_hi = np.tile(q_scale[64:], 2)[:, None]
    ks_lo = np.tile(k_scale[:64], 2)[:, None]
    ks_hi = np.tile(k_scale[64:], 2)[:, None]
    rtq_b, rtk_b = [], []
    for b in range(B):
        pos = _positions(seg[b])
        sinus = pos[None, :] / timescale[:, None]        # [64, T]
        sd = np.vstack([np.sin(sinus)] * 2)              # [128, T]
        cd = np.vstack([np.cos(sinus)] * 2)
        rtq_b.append(np.hstack([qs_lo * cd, qs_hi * sd, qs_hi * cd, qs_lo * sd]
                               ).astype(bf))
        rtk_b.append(np.hstack([ks_lo * cd, ks_hi * sd, ks_hi * cd, ks_lo * sd]
                               ).astype(bf))

    cstb = np.zeros((128, 4), np.float32)
    cstb[0:64, 0] = 1.0
    cstb[64:128, 1] = 1.0
    cstb[:, 2] = 1.0
    cst = np.zeros((128, 2), np.float32)
    cst[:, 0] = H * EPS
    cst[:, 1] = EPS
    bcw = np.zeros((2, 128), np.float32)
    bcw[0, 0:64] = 1.0
    bcw[1, 64:128] = 1.0
    iden = np.eye(128, dtype=np.float32)

    in_maps = []
    for core in range(8):
        b, kv = core // K, core % K
        xt = np.ascontiguousarray(
            x[b].T.reshape(ND, 128, T).transpose(1, 0, 2).reshape(128, ND * T))

        def qfeat(w, cols):
            # [D, 128] -> [128(d_lo), ND*128] with w[p, di*128+j] = W[di*128+p, cols[j]]
            sub = w[:, cols]                             # [D, 128]
            return sub.reshape(ND, 128, 128).transpose(1, 0, 2).reshape(128, D)

        base = kv * 4 * H
        f_cols = []
        for pair in range(2):      # (f0,f2) then (f1,f3)
            for hv in range(2):
                cols = np.concatenate([
                    np.arange(base + (2 * g4 + pair) * H + hv * 64,
                              base + (2 * g4 + pair) * H + hv * 64 + 64)
                    for g4 in range(2)])
                f_cols.append(cols)
        # heads order per pair: pair0 -> heads (0,1)?  cols above pick heads
        # (pair + 2*g4): pair0 -> heads 0,2 ... fix: want pair0 = heads 0,1.
        f_cols = []
        for pair, heads in [(0, (0, 1)), (1, (2, 3))]:
            for hv in range(2):
                cols = np.concatenate([
                    np.arange(base + g4 * H + hv * 64,
                              base + g4 * H + hv * 64 + 64) for g4 in heads])
                f_cols.append(cols)
        wq0 = np.hstack([qfeat(Wq, f_cols[0]), qfeat(Wq, f_cols[1])]).astype(bf)
        wq1 = np.hstack([qfeat(Wq, f_cols[2]), qfeat(Wq, f_cols[3])]).astype(bf)
        kcols = np.arange(kv * H, (kv + 1) * H)
        wkv = np.hstack([qfeat(Wk, kcols), qfeat(Wv, kcols)]).astype(bf)
        wo_t = np.ascontiguousarray(
            Wo[kv * 512:(kv + 1) * 512].reshape(G, 128, D)
            .transpose(1, 0, 2).reshape(128, G * D)).astype(bf)
        nm = max(masks[b].shape[0], 1)
        mpk = np.ascontiguousarray(
            masks[b].transpose(1, 0, 2).reshape(128, nm * 128)).astype(bf)

        in_maps.append({
            "xt": xt.astype(bf), "wq0": wq0, "wq1": wq1, "wkv": wkv,
            "wo": wo_t, "rtq": rtq_b[b], "rtk": rtk_b[b], "mpk": mpk,
            "iden": iden.astype(bf), "cstb": cstb.astype(bf),
            "cst": cst, "bcw": bcw,
        })

    do_trace = os.environ.get("BASS_TRACE") == "1"
    res = run_bass_kernel_spmd(
        nc, in_maps, core_ids=list(range(8)), trace=do_trace)
    LAST_RESULTS = res

    out = np.zeros((B, T, D), np.float32)
    for core in range(8):
        out[core // K] += res.results[core]["out"].astype(np.float32)
    return out
